# revision 62
# baseline (speedup 1.0000x reference)
"""Trainium2 Bass kernel for AdaptiveInterpolationModule (dual-source cross-attention).

Reference computation (B=16, S=1024, D=768):
    Q   = x_C @ W_q.T + b_q
    K_s = x_s @ W_k.T + b_k          (s in {A, B})
    V_s = x_s @ W_v.T + b_v
    attn_s   = softmax(Q K_s^T / sqrt(D))
    interp_s = attn_s V_s
    h   = LayerNorm(interp_A + interp_B + x_C) * gamma + beta
    out = h @ W_fc.T + b_fc

Sharding: data-parallel over batch, 2 batches per core on 8 cores. No collectives.

Math simplifications (exact):
  - scores = Q K_s^T = x_C (W_q^T W_k) x_s^T + (b_q W_k) x_s^T: the Q, K_A and
    K_B projections collapse into ONE projection G^T = (W_q^T W_k)^T x_C^T,
    and the scores matmuls take the already-resident x_s^T tiles as the
    stationary operand directly.
  - b_k never affects the output: scores rows shift by a k-constant -> softmax invariant.
  - b_v contributes exactly +b_v per source (attn rows sum to 1) -> folded into the
    residual input on the host (x_C + 2*b_v).
  - the b_q W_k x_s^T score term is a per-KEY constant: host-computed and fed
    to the Exp activation as a per-partition bias AP (with -ESHIFT folded in);
    the softmax 1/sqrt(D) lives inside the Exp activation's scale.
  - LayerNorm's gamma/beta folded into the fc layer on the host:
    out = h @ (W_fc*gamma).T + (b_fc + W_fc@beta), with h = (t1-mu)*rstd
    applied on the ACT engine while casting to bf16 (scale/bias are
    per-partition APs), so the post-fc copyout is a single +b_fc add.
  - softmax computed without per-row max subtraction: a constant shift ESHIFT keeps
    exp() inside fp8e4m3 range (shift-invariant, exact).

Precision (gate 2e-2):
  - f32: PSUM accumulation, softmax row-sums/normalization, LayerNorm, residual, output.
  - bf16: fc inputs (h and W_fc) -- the accuracy anchor (errors here pass
    straight to the output; attention-path errors average out over keys).
  - fp8e4m3 (+DoubleRow, K=256/matmul -> half the PE passes): all Q/K/V
    projection inputs, Q^T/K^T storage (scores), exp(scores) and V (interp).

Device dataflow per (core, batch):
  - x^T tiles (pre-transposed on host) + W^T tiles (pre-transposed on host).
  - Q^T, K_A^T, K_B^T computed transposed [e, s]; V_A, V_B computed natural [s, e]
    with two extra all-ones columns so the interp matmul also produces softmax
    row-sums for free.
  - scores^T[k, q] per q-block of 512: k on partitions, so softmax needs no
    transposes at all; exp PSUM->SBUF fp8 on ACT.
  - interp accumulated over both sources' k-chunks; 1/rowsum applied as per-partition
    scalars during PSUM->SBUF copyout with the residual fused in; LayerNorm
    (bn_stats/bn_aggr); PE-transpose z in bf16 pairs; fc matmul; +b_fc; out.
  - Per q-block the epilogues are emitted in two passes (all interp+LN, then all
    transpose+fc) so the PE's static order never waits on a DVE chain.
  - dma_start issue costs ~600ns of the issuing sequencer, so initial loads are
    split per d-chunk across the sync and gpsimd lanes, critical tensors first.
"""

import sys

import numpy as np

try:
    import concourse.bass as bass
except ImportError:
    sys.path.insert(0, "/opt/trn_rl_repo")
    import concourse.bass as bass

import ml_dtypes
from contextlib import ExitStack

import concourse.mybir as mybir
import concourse.tile as tile
from concourse import bacc
from concourse.bass_utils import run_bass_kernel_spmd
from concourse.masks import make_identity

P = 128
DIM = 768
S = 1024
B = 16
NCORES = 8
BPC = B // NCORES  # batches per core
DCH = DIM // P     # 6 chunks of 128 along D
SCH = S // P       # 8 chunks of 128 along S
EPS = 1e-5
SCALE = 1.0 / float(np.sqrt(DIM))
F32 = mybir.dt.float32
BF16 = mybir.dt.bfloat16
FP8 = mybir.dt.float8e4
# constant softmax shift: exp(score - ESHIFT) keeps values inside fp8e4m3
# range (max score on N(0,1)-scaled data is ~8 -> exp(4.5) = 90 < 448).
# Softmax is shift-invariant so this is exact.
ESHIFT = 3.5

AF = mybir.ActivationFunctionType
ALU = mybir.AluOpType

VW = DIM + 2  # V tile width: 768 value cols + 2 ones cols (row-sum trick)


def build_bass() -> bass.Bass:
    nc = bacc.Bacc()

    xaT = nc.declare_dram_parameter("xaT", [BPC, DIM, S], FP8, isOutput=False)
    xbT = nc.declare_dram_parameter("xbT", [BPC, DIM, S], FP8, isOutput=False)
    xcT = nc.declare_dram_parameter("xcT", [BPC, DIM, S], FP8, isOutput=False)
    xcr = nc.declare_dram_parameter("xcr", [BPC, S, DIM], BF16, isOutput=False)
    # wm = W_q^T @ W_k: scores = x_C @ wm @ x_s^T, so Q/K_A/K_B projections
    # collapse into ONE projection G^T = wm^T x_C^T; the b_q @ W_k @ x_s^T
    # score term is precomputed on the host as a per-key exp bias (ck).
    wm = nc.declare_dram_parameter("wm", [DIM, DIM], FP8, isOutput=False)
    wvT = nc.declare_dram_parameter("wvT", [DIM, DIM], FP8, isOutput=False)
    wfT = nc.declare_dram_parameter("wfT", [DIM, DIM], BF16, isOutput=False)
    ck = nc.declare_dram_parameter("ck", [BPC, P, 2, SCH], F32, isOutput=False)
    bfc = nc.declare_dram_parameter("bfc", [DIM], BF16, isOutput=False)
    out = nc.declare_dram_parameter("out", [BPC, S, DIM], BF16, isOutput=True)

    with tile.TileContext(nc) as tc, ExitStack() as ctx:
        consts = ctx.enter_context(tc.tile_pool(name="consts", bufs=1))
        wpool = ctx.enter_context(tc.tile_pool(name="wpool", bufs=1))
        xpool = ctx.enter_context(tc.tile_pool(name="xpool", bufs=1))
        qkv = ctx.enter_context(tc.tile_pool(name="qkv", bufs=1))
        epool = ctx.enter_context(tc.tile_pool(name="epool", bufs=2))
        spool = ctx.enter_context(tc.tile_pool(name="spool", bufs=4))
        zpool = ctx.enter_context(tc.tile_pool(name="zpool", bufs=5))
        opool = ctx.enter_context(tc.tile_pool(name="opool", bufs=4))
        # one bank-sized psum pool: 6 of the 8 banks rotate through all
        # matmul outputs (projections, scores, interp 385-slices, fc); the
        # remaining 2 banks hold the transpose pairs.
        ps512 = ctx.enter_context(tc.tile_pool(name="ps512", bufs=6, space="PSUM"))
        pstr = ctx.enter_context(tc.tile_pool(name="pstr", bufs=2, space="PSUM"))

        # --- first-needed DMAs first: the K_A projection needs xaT + wkT.
        # dma_start issue costs ~600ns of the issuing engine's sequencer, so
        # spread the initial loads across otherwise-idle sequencers; the very
        # first tensors (xa, wk) go 4-wide so the PE can start sooner.
        dma_lanes = [nc.sync, nc.gpsimd]
        boot_lanes = [nc.sync, nc.gpsimd, nc.scalar]

        def load_xT(h, b, tag, lane0=0, dt=BF16, lanes=None):
            lanes = lanes or dma_lanes
            t = xpool.tile([P, DCH, S], dt, tag=tag)
            v = h[b].rearrange("(o p) s -> p o s", p=P)
            for do in range(DCH):
                eng = lanes[(lane0 + do) % len(lanes)]
                eng.dma_start(out=t[:, do, :], in_=v[:, do, :])
            return t

        def load_wT(h, tag, lane0=0, dt=BF16, lanes=None):
            lanes = lanes or dma_lanes
            t = wpool.tile([P, DCH, DIM], dt, tag=tag)
            v = h[:].rearrange("(o p) e -> p o e", p=P)
            for do in range(DCH):
                eng = lanes[(lane0 + do) % len(lanes)]
                eng.dma_start(out=t[:, do, :], in_=v[:, do, :])
            return t

        # The G projection's inputs gate the first matmul: interleave the
        # xc/wm chunk DMAs 3 queues wide so the first DR pair (d0,d1 of both)
        # lands after ~4 transfers instead of after all 12.
        xc_b0 = xpool.tile([P, DCH, S], FP8, tag="xcT")
        wm_t = wpool.tile([P, DCH, DIM], FP8, tag="wm")
        xcv = xcT[0].rearrange("(o p) s -> p o s", p=P)
        wmv = wm[:].rearrange("(o p) e -> p o e", p=P)
        for do in range(DCH):
            boot_lanes[(2 * do) % 3].dma_start(out=xc_b0[:, do, :], in_=xcv[:, do, :])
            boot_lanes[(2 * do + 1) % 3].dma_start(out=wm_t[:, do, :], in_=wmv[:, do, :])
        w_sb = {"m": wm_t}
        xa_b0 = load_xT(xaT, 0, "xaT", lane0=0, dt=FP8)
        w_sb["v"] = load_wT(wvT, "wv", lane0=1, dt=FP8)
        xb_b0 = load_xT(xbT, 0, "xbT", lane0=0, dt=FP8)
        w_sb["f"] = load_wT(wfT, "wf", lane0=1)

        bfc_sb = consts.tile([P, DIM], BF16)
        nc.sync.dma_start(out=bfc_sb, in_=bfc[:].partition_broadcast(P))

        idn = consts.tile([P, P], BF16)
        make_identity(nc, idn)
        eps_sb = consts.tile([P, 1], F32)
        nc.vector.memset(eps_sb, EPS)

        for b in range(BPC):
            if b == 0:
                x_sb = {"a": xa_b0, "b": xb_b0, "c": xc_b0}
            else:
                x_sb = {
                    "c": load_xT(xcT, b, "xcT", lane0=0, dt=FP8),
                    "a": load_xT(xaT, b, "xaT", lane0=2, dt=FP8),
                    "b": load_xT(xbT, b, "xbT", lane0=1, dt=FP8),
                }
            # per-key exp bias: SCALE*(x_s @ (W_k^T b_q)) - ESHIFT
            ck_sb = xpool.tile([P, 2, SCH], F32, tag="ck")
            nc.gpsimd.dma_start(out=ck_sb, in_=ck[b])

            # --- projections Q^T, K_A^T, K_B^T: [e, s] (e on partitions),
            # stored fp8 (values ~N(0,1), well inside e4m3 range); the softmax
            # 1/sqrt(D) scale is applied later inside the Exp activation ---
            def projT(tag, w_t, x_t, bias_ap=None, dr=False):
                dst = qkv.tile([P, DCH, S], FP8, tag=tag)
                for ec in range(DCH):
                    for sh in range(S // 512):
                        ps = ps512.tile([P, 512], F32, tag="ps512")
                        if dr:
                            for dp in range(DCH // 2):
                                dsl = slice(2 * dp, 2 * dp + 2)
                                nc.tensor.matmul(
                                    ps,
                                    lhsT=w_t[:, dsl, ec * P:(ec + 1) * P],
                                    rhs=x_t[:, dsl, sh * 512:(sh + 1) * 512],
                                    start=(dp == 0),
                                    stop=(dp == DCH // 2 - 1),
                                    perf_mode=mybir.MatmulPerfMode.DoubleRow,
                                )
                        else:
                            for do in range(DCH):
                                nc.tensor.matmul(
                                    ps,
                                    lhsT=w_t[:, do, ec * P:(ec + 1) * P],
                                    rhs=x_t[:, do, sh * 512:(sh + 1) * 512],
                                    start=(do == 0),
                                    stop=(do == DCH - 1),
                                )
                        o = dst[:, ec, sh * 512:(sh + 1) * 512]
                        if bias_ap is not None:
                            nc.scalar.activation(
                                out=o, in_=ps, func=AF.Identity,
                                bias=bias_ap[:, ec:ec + 1], scale=1.0,
                            )
                        else:
                            # alternate the PSUM->SBUF casts across DVE and
                            # ACT (Pool can't read PSUM) so neither engine
                            # gates psum reuse
                            if (ec * 2 + sh) % 2 == 0:
                                nc.vector.tensor_copy(out=o, in_=ps)
                            else:
                                nc.scalar.copy(out=o, in_=ps)
                return dst

            gT_sb = projT("GT", w_sb["m"], x_sb["c"], dr=True)

            # --- V_A, V_B natural layout [s, e] + two ones columns ---
            v_sb = {}
            for name in ("a", "b"):
                dst = qkv.tile([P, SCH, VW], FP8, tag=f"V{name.upper()}")
                nc.vector.memset(dst[:, :, DIM:VW], 1.0)
                for sc in range(SCH):
                    for off, w in ((0, 384), (384, 384)):
                        ps = ps512.tile([P, 512], F32, tag="ps512")
                        for dp in range(DCH // 2):
                            dsl = slice(2 * dp, 2 * dp + 2)
                            nc.tensor.matmul(
                                ps[:, :w],
                                lhsT=x_sb[name][:, dsl, sc * P:(sc + 1) * P],
                                rhs=w_sb["v"][:, dsl, off:off + w],
                                start=(dp == 0),
                                stop=(dp == DCH // 2 - 1),
                                perf_mode=mybir.MatmulPerfMode.DoubleRow,
                            )
                        if (sc + (0 if off else 1)) % 2 == 0:
                            nc.vector.tensor_copy(out=dst[:, sc, off:off + w], in_=ps[:, :w])
                        else:
                            nc.scalar.copy(out=dst[:, sc, off:off + w], in_=ps[:, :w])
                v_sb[name] = dst

            # --- attention + epilogue, per q-block of 512 ---
            for qb in range(S // 512):
                qsl = slice(qb * 512, (qb + 1) * 512)
                # scores^T and exp: e^T[k, q] = exp(x_s[k,:] @ G[q,:] + ck)
                # with x_s^T itself as the stationary operand (no K tiles).
                e_sb = {}
                for si, name in enumerate(("a", "b")):
                    et = epool.tile([P, SCH, 512], FP8, tag=f"e{name.upper()}")
                    for kc in range(SCH):
                        ps = ps512.tile([P, 512], F32, tag="ps512")
                        for ep in range(DCH // 2):
                            esl = slice(2 * ep, 2 * ep + 2)
                            nc.tensor.matmul(
                                ps,
                                lhsT=x_sb[name][:, esl, kc * P:(kc + 1) * P],
                                rhs=gT_sb[:, esl, qsl],
                                start=(ep == 0),
                                stop=(ep == DCH // 2 - 1),
                                perf_mode=mybir.MatmulPerfMode.DoubleRow,
                            )
                        # exp(score/sqrt(D) + SCALE*b_q.W_k.x_s[k] - ESHIFT)
                        nc.scalar.activation(
                            out=et[:, kc, :], in_=ps, func=AF.Exp,
                            bias=ck_sb[:, si, kc:kc + 1], scale=SCALE,
                        )
                    e_sb[name] = et

                # stage 1 (per 128-row tile): interp + layernorm -> z[qi]
                def stage1(qi):
                    qc = qb * 4 + qi
                    qs = slice(qi * P, (qi + 1) * P)

                    xc_t = opool.tile([P, DIM], BF16, tag="xc")
                    nc.gpsimd.dma_start(out=xc_t, in_=xcr[b, qc * P:(qc + 1) * P, :])

                    # interp psums, split 385/385 so every PE pass is longer
                    # than a LDWEIGHTS (135ns); h1 carries the ones columns
                    # -> row-sums at p1 col 383 (= v col 768)
                    pa = {}
                    for name in ("a", "b"):
                        p0 = ps512.tile([P, 512], F32, tag="ps512")
                        p1 = ps512.tile([P, 512], F32, tag="ps512")
                        for kp in range(SCH // 2):
                            ksl = slice(2 * kp, 2 * kp + 2)
                            nc.tensor.matmul(
                                p0[:, 0:385],
                                lhsT=e_sb[name][:, ksl, qs],
                                rhs=v_sb[name][:, ksl, 0:385],
                                start=(kp == 0),
                                stop=(kp == SCH // 2 - 1),
                                perf_mode=mybir.MatmulPerfMode.DoubleRow,
                            )
                        for kp in range(SCH // 2):
                            ksl = slice(2 * kp, 2 * kp + 2)
                            nc.tensor.matmul(
                                p1[:, 0:385],
                                lhsT=e_sb[name][:, ksl, qs],
                                rhs=v_sb[name][:, ksl, 385:VW],
                                start=(kp == 0),
                                stop=(kp == SCH // 2 - 1),
                                perf_mode=mybir.MatmulPerfMode.DoubleRow,
                            )
                        pa[name] = (p0, p1)

                    rcp = {}
                    for name in ("a", "b"):
                        r = spool.tile([P, 1], F32, tag=f"r{name}")
                        nc.vector.reciprocal(r, pa[name][1][:, 383:384])
                        rcp[name] = r

                    # t1 = psA*rA + xc ; t1 += psB*rB   (residual fused)
                    t1 = spool.tile([P, DIM], F32, tag="t1")
                    for (off, w, pi) in ((0, 385, 0), (385, 383, 1)):
                        nc.vector.scalar_tensor_tensor(
                            out=t1[:, off:off + w],
                            in0=pa["a"][pi][:, 0:w],
                            scalar=rcp["a"], in1=xc_t[:, off:off + w],
                            op0=ALU.mult, op1=ALU.add,
                        )
                        nc.vector.scalar_tensor_tensor(
                            out=t1[:, off:off + w],
                            in0=pa["b"][pi][:, 0:w],
                            scalar=rcp["b"], in1=t1[:, off:off + w],
                            op0=ALU.mult, op1=ALU.add,
                        )

                    # layernorm
                    stats = spool.tile([P, 3, 6], F32, tag="st")
                    for g in range(3):
                        nc.vector.bn_stats(
                            out=stats[:, g, :], in_=t1[:, g * 256:(g + 1) * 256]
                        )
                    mv = spool.tile([P, 2], F32, tag="mv")
                    nc.vector.bn_aggr(out=mv, in_=stats)
                    std = spool.tile([P, 1], F32, tag="std")
                    nc.scalar.activation(
                        out=std, in_=mv[:, 1:2], func=AF.Sqrt, bias=eps_sb
                    )
                    rstd = spool.tile([P, 1], F32, tag="rstd")
                    nc.vector.reciprocal(rstd, std)
                    # z = (t1 - mu) * rstd, computed on ACT as
                    # Identity(t1 * rstd + (-mu * rstd)); LayerNorm is now
                    # fully applied BEFORE fc, so the fc copyout is a plain
                    # +b_fc and no mean/std correction is needed after.
                    nmr = spool.tile([P, 1], F32, tag="nmr")
                    nc.vector.tensor_scalar(
                        nmr, mv[:, 0:1], -1.0, rstd, ALU.mult, ALU.mult
                    )
                    # z split DVE/ACT so the transposes (which wait on z)
                    # see ~half the latency
                    z = zpool.tile([P, DIM], BF16, tag="z")
                    nc.vector.tensor_scalar(
                        z[:, 0:256], t1[:, 0:256], rstd, nmr, ALU.mult, ALU.add
                    )
                    nc.scalar.activation(
                        out=z[:, 256:DIM], in_=t1[:, 256:DIM], func=AF.Identity,
                        bias=nmr, scale=rstd,
                    )
                    return z

                # stage 2: transpose h + fc + store
                def stage2(qi, z):
                    qc = qb * 4 + qi

                    hT = opool.tile([P, DCH, P], BF16, tag="hT")
                    for ep in range(DCH // 2):
                        pst = pstr.tile([P, 2, P], BF16, tag="pstr")
                        for j in range(2):
                            eo = ep * 2 + j
                            nc.tensor.transpose(
                                pst[:, j], z[:, eo * P:(eo + 1) * P], idn
                            )
                        nc.scalar.copy(out=hT[:, ep * 2:(ep + 1) * 2, :], in_=pst)

                    o_t = opool.tile([P, DIM], BF16, tag="o")
                    for off, w in ((0, 384), (384, 384)):
                        ps = ps512.tile([P, 512], F32, tag="ps512")
                        for eo in range(DCH):
                            nc.tensor.matmul(
                                ps[:, :w],
                                lhsT=hT[:, eo, :],
                                rhs=w_sb["f"][:, eo, off:off + w],
                                start=(eo == 0),
                                stop=(eo == DCH - 1),
                            )
                        nc.vector.scalar_tensor_tensor(
                            out=o_t[:, off:off + w],
                            in0=ps[:, :w], scalar=0.0,
                            in1=bfc_sb[:, off:off + w],
                            op0=ALU.bypass, op1=ALU.add,
                        )
                        # per-chunk store: the last tile's DMA starts after
                        # the first chunk's add instead of after the whole row
                        nc.sync.dma_start(
                            out=out[b, qc * P:(qc + 1) * P, off:off + w],
                            in_=o_t[:, off:off + w],
                        )

                # software pipeline: PE's stage-2 work for tile qi fills the
                # time the DVE needs to drain tile qi+1's interp psums, so
                # the interp matmuls never wait a full STT chain.
                z0 = stage1(0)
                z1 = stage1(1)
                stage2(0, z0)
                z2 = stage1(2)
                stage2(1, z1)
                z3 = stage1(3)
                stage2(2, z2)
                stage2(3, z3)

    nc.compile()
    return nc


_CACHED_NC = None
_LAST_IN_MAPS = None


def kernel(**inputs) -> np.ndarray:
    global _CACHED_NC, _LAST_IN_MAPS
    bf16 = ml_dtypes.bfloat16
    f32 = np.float32

    xA = np.asarray(inputs["x_A"], dtype=f32)
    xB = np.asarray(inputs["x_B"], dtype=f32)
    xC = np.asarray(inputs["x_C"], dtype=f32)

    fp8 = ml_dtypes.float8_e4m3
    xaT = np.ascontiguousarray(xA.transpose(0, 2, 1)).astype(fp8)
    xbT = np.ascontiguousarray(xB.transpose(0, 2, 1)).astype(fp8)
    xcT = np.ascontiguousarray(xC.transpose(0, 2, 1)).astype(fp8)
    xcr = (xC + 2.0 * np.asarray(inputs["b_v"], dtype=f32)).astype(bf16)

    W_q = np.asarray(inputs["W_q"], dtype=f32)
    W_k = np.asarray(inputs["W_k"], dtype=f32)
    b_q = np.asarray(inputs["b_q"], dtype=f32)
    # scores = Q K^T = x_C (W_q^T W_k) x_s^T + (b_q W_k) x_s^T
    wm = np.ascontiguousarray(W_q.T @ W_k).astype(fp8)
    vk = b_q @ W_k
    # per-key exp bias, laid out [b, p, src, kc] so the DMA is contiguous
    ck_full = np.stack(
        [SCALE * (xA @ vk) - ESHIFT, SCALE * (xB @ vk) - ESHIFT], axis=1
    )  # [B, 2, S]
    ck_arr = np.ascontiguousarray(
        ck_full.reshape(B, 2, S // P, P).transpose(0, 3, 1, 2)
    ).astype(f32)  # [B, P, 2, SCH]
    wvT = np.ascontiguousarray(np.asarray(inputs["W_v"], dtype=f32).T).astype(fp8)

    # fold LayerNorm's gamma/beta into the fc layer (exact):
    #   h = z*gamma + beta;  out = h @ W_fc.T + b_fc
    #     = z @ (W_fc * gamma).T + (b_fc + W_fc @ beta)
    gam = np.asarray(inputs["gamma"], dtype=f32)
    bet = np.asarray(inputs["beta"], dtype=f32)
    W_fc = np.asarray(inputs["W_fc"], dtype=f32)
    wfT = np.ascontiguousarray(W_fc.T * gam[:, None]).astype(bf16)
    bfc = (np.asarray(inputs["b_fc"], dtype=f32) + W_fc @ bet).astype(bf16)

    if _CACHED_NC is None:
        _CACHED_NC = build_bass()
    nc = _CACHED_NC

    in_maps = []
    for c in range(NCORES):
        sl = slice(c * BPC, (c + 1) * BPC)
        in_maps.append({
            "xaT": np.ascontiguousarray(xaT[sl]),
            "xbT": np.ascontiguousarray(xbT[sl]),
            "xcT": np.ascontiguousarray(xcT[sl]),
            "xcr": np.ascontiguousarray(xcr[sl]),
            "wm": wm, "wvT": wvT, "wfT": wfT,
            "ck": np.ascontiguousarray(ck_arr[sl]), "bfc": bfc,
        })

    _LAST_IN_MAPS = in_maps
    res = run_bass_kernel_spmd(nc, in_maps, core_ids=list(range(NCORES)))
    outs = [np.asarray(res.results[i]["out"], dtype=f32) for i in range(NCORES)]
    return np.concatenate(outs, axis=0)


if __name__ == "__main__":
    rng = np.random.default_rng(0)
    fake = {
        "x_A": rng.standard_normal((B, S, DIM), dtype=np.float32),
        "x_B": rng.standard_normal((B, S, DIM), dtype=np.float32),
        "x_C": rng.standard_normal((B, S, DIM), dtype=np.float32),
        "W_q": rng.standard_normal((DIM, DIM), dtype=np.float32) / 27.7,
        "b_q": rng.standard_normal(DIM).astype(np.float32) / 27.7,
        "W_k": rng.standard_normal((DIM, DIM), dtype=np.float32) / 27.7,
        "b_k": rng.standard_normal(DIM).astype(np.float32) / 27.7,
        "W_v": rng.standard_normal((DIM, DIM), dtype=np.float32) / 27.7,
        "b_v": rng.standard_normal(DIM).astype(np.float32) / 27.7,
        "gamma": np.ones(DIM, np.float32),
        "beta": np.zeros(DIM, np.float32),
        "W_fc": rng.standard_normal((DIM, DIM), dtype=np.float32) / 27.7,
        "b_fc": rng.standard_normal(DIM).astype(np.float32) / 27.7,
    }
    o = kernel(**fake)
    print(o.shape, o.dtype)



# revision 65
# speedup vs baseline: 1.1659x; 1.1659x over previous
"""Trainium2 Bass kernel for AdaptiveInterpolationModule (dual-source cross-attention).

Reference computation (B=16, S=1024, D=768):
    Q   = x_C @ W_q.T + b_q
    K_s = x_s @ W_k.T + b_k          (s in {A, B})
    V_s = x_s @ W_v.T + b_v
    attn_s   = softmax(Q K_s^T / sqrt(D))
    interp_s = attn_s V_s
    h   = LayerNorm(interp_A + interp_B + x_C) * gamma + beta
    out = h @ W_fc.T + b_fc

Sharding: data-parallel over batch, 2 batches per core on 8 cores. No collectives.

Math simplifications (exact):
  - scores = Q K_s^T = x_C (W_q^T W_k) x_s^T + (b_q W_k) x_s^T: the Q, K_A and
    K_B projections collapse into ONE projection G^T = (W_q^T W_k)^T x_C^T,
    and the scores matmuls take the already-resident x_s^T tiles as the
    stationary operand directly.
  - b_k never affects the output: scores rows shift by a k-constant -> softmax invariant.
  - b_v contributes exactly +b_v per source (attn rows sum to 1) -> folded into the
    residual input on the host (x_C + 2*b_v).
  - the b_q W_k x_s^T score term is a per-KEY constant: host-computed and fed
    to the Exp activation as a per-partition bias AP (with -ESHIFT folded in);
    the softmax 1/sqrt(D) lives inside the Exp activation's scale.
  - LayerNorm's gamma/beta folded into the fc layer on the host:
    out = h @ (W_fc*gamma).T + (b_fc + W_fc@beta), with h = (t1-mu)*rstd
    applied on the ACT engine while casting to bf16 (scale/bias are
    per-partition APs), so the post-fc copyout is a single +b_fc add.
  - softmax computed without per-row max subtraction: a constant shift ESHIFT keeps
    exp() inside fp8e4m3 range (shift-invariant, exact).

Precision (gate 2e-2):
  - f32: PSUM accumulation, softmax row-sums/normalization, LayerNorm, residual, output.
  - bf16: fc inputs (h and W_fc) -- the accuracy anchor (errors here pass
    straight to the output; attention-path errors average out over keys).
  - fp8e4m3 (+DoubleRow, K=256/matmul -> half the PE passes): all Q/K/V
    projection inputs, Q^T/K^T storage (scores), exp(scores) and V (interp).

Device dataflow per (core, batch):
  - x^T tiles (pre-transposed on host) + W^T tiles (pre-transposed on host).
  - G^T computed transposed [e, s]; V_A, V_B computed natural [s, e]
    with two extra all-ones columns so the interp matmul also produces softmax
    row-sums for free.
  - scores^T[k, q] per q-block of 512: k on partitions, so softmax needs no
    transposes at all; exp PSUM->SBUF fp8 on ACT.
  - interp accumulated over both sources' k-chunks; 1/rowsum applied as per-partition
    scalars during PSUM->SBUF copyout with the residual fused in; LayerNorm
    (bn_stats/bn_aggr); PE-transpose z in bf16 pairs; fc matmul; +b_fc; out.
  - Per q-block the epilogues are emitted in two passes (all interp+LN, then all
    transpose+fc) so the PE's static order never waits on a DVE chain.
  - dma_start issue costs ~600ns of the issuing sequencer, so initial loads are
    split per d-chunk across the sync and gpsimd lanes, critical tensors first.
"""

import sys

import numpy as np

try:
    import concourse.bass as bass
except ImportError:
    sys.path.insert(0, "/opt/trn_rl_repo")
    import concourse.bass as bass

import ml_dtypes
from contextlib import ExitStack

import concourse.mybir as mybir
import concourse.tile as tile
from concourse import bacc
from concourse.bass_utils import run_bass_kernel_spmd
from concourse.masks import make_identity

P = 128
DIM = 768
S = 1024
B = 16
NCORES = 8
BPC = B // NCORES  # batches per core
DCH = DIM // P     # 6 chunks of 128 along D
SCH = S // P       # 8 chunks of 128 along S
EPS = 1e-5
SCALE = 1.0 / float(np.sqrt(DIM))
F32 = mybir.dt.float32
BF16 = mybir.dt.bfloat16
FP8 = mybir.dt.float8e4
# constant softmax shift: exp(score - ESHIFT) keeps values inside fp8e4m3
# range (max score on N(0,1)-scaled data is ~8 -> exp(4.5) = 90 < 448).
# Softmax is shift-invariant so this is exact.
ESHIFT = 3.5

AF = mybir.ActivationFunctionType
ALU = mybir.AluOpType

VW = DIM + 2  # V tile width: 768 value cols + 2 ones cols (row-sum trick)


def build_bass() -> bass.Bass:
    nc = bacc.Bacc()

    xaT = nc.declare_dram_parameter("xaT", [BPC, DIM, S], FP8, isOutput=False)
    xbT = nc.declare_dram_parameter("xbT", [BPC, DIM, S], FP8, isOutput=False)
    xcT = nc.declare_dram_parameter("xcT", [BPC, DIM, S], FP8, isOutput=False)
    xcr = nc.declare_dram_parameter("xcr", [BPC, S, DIM], BF16, isOutput=False)
    # wm = W_q^T @ W_k: scores = x_C @ wm @ x_s^T, so Q/K_A/K_B projections
    # collapse into ONE projection G^T = wm^T x_C^T; the b_q @ W_k @ x_s^T
    # score term is precomputed on the host as a per-key exp bias (ck).
    wm = nc.declare_dram_parameter("wm", [DIM, DIM], FP8, isOutput=False)
    wvT = nc.declare_dram_parameter("wvT", [DIM, DIM], FP8, isOutput=False)
    wfT = nc.declare_dram_parameter("wfT", [DIM, DIM], BF16, isOutput=False)
    ck = nc.declare_dram_parameter("ck", [BPC, P, 2, SCH], F32, isOutput=False)
    bfc = nc.declare_dram_parameter("bfc", [DIM], BF16, isOutput=False)
    out = nc.declare_dram_parameter("out", [BPC, S, DIM], BF16, isOutput=True)

    with tile.TileContext(nc) as tc, ExitStack() as ctx:
        consts = ctx.enter_context(tc.tile_pool(name="consts", bufs=1))
        wpool = ctx.enter_context(tc.tile_pool(name="wpool", bufs=1))
        xpool = ctx.enter_context(tc.tile_pool(name="xpool", bufs=1))
        qkv = ctx.enter_context(tc.tile_pool(name="qkv", bufs=1))
        epool = ctx.enter_context(tc.tile_pool(name="epool", bufs=2))
        spool = ctx.enter_context(tc.tile_pool(name="spool", bufs=4))
        zpool = ctx.enter_context(tc.tile_pool(name="zpool", bufs=5))
        opool = ctx.enter_context(tc.tile_pool(name="opool", bufs=4))
        # one bank-sized psum pool: 6 of the 8 banks rotate through all
        # matmul outputs (projections, scores, interp 385-slices, fc); the
        # remaining 2 banks hold the transpose pairs.
        ps512 = ctx.enter_context(tc.tile_pool(name="ps512", bufs=6, space="PSUM"))
        pstr = ctx.enter_context(tc.tile_pool(name="pstr", bufs=2, space="PSUM"))

        # --- first-needed DMAs first: the G projection needs xcT + wm.
        # dma_start issue costs ~600ns of the issuing engine's sequencer, so
        # spread the initial loads across otherwise-idle sequencers; the very
        # first tensors (xc, wm) go 3-wide so the PE can start sooner.
        dma_lanes = [nc.sync, nc.gpsimd]
        boot_lanes = [nc.sync, nc.gpsimd, nc.scalar]

        def load_xT(h, b, tag, lane0=0, dt=BF16, lanes=None):
            lanes = lanes or dma_lanes
            t = xpool.tile([P, DCH, S], dt, tag=tag)
            v = h[b].rearrange("(o p) s -> p o s", p=P)
            for do in range(DCH):
                eng = lanes[(lane0 + do) % len(lanes)]
                eng.dma_start(out=t[:, do, :], in_=v[:, do, :])
            return t

        def load_wT(h, tag, lane0=0, dt=BF16, lanes=None):
            lanes = lanes or dma_lanes
            t = wpool.tile([P, DCH, DIM], dt, tag=tag)
            v = h[:].rearrange("(o p) e -> p o e", p=P)
            for do in range(DCH):
                eng = lanes[(lane0 + do) % len(lanes)]
                eng.dma_start(out=t[:, do, :], in_=v[:, do, :])
            return t

        # The G projection's inputs gate the first matmul: interleave the
        # xc/wm chunk DMAs 3 queues wide so the first DR pair (d0,d1 of both)
        # lands after ~4 transfers instead of after all 12.
        xc_b0 = xpool.tile([P, DCH, S], FP8, tag="xcT")
        wm_t = wpool.tile([P, DCH, DIM], FP8, tag="wm")
        xcv = xcT[0].rearrange("(o p) s -> p o s", p=P)
        wmv = wm[:].rearrange("(o p) e -> p o e", p=P)
        for do in range(DCH):
            boot_lanes[(2 * do) % 3].dma_start(out=xc_b0[:, do, :], in_=xcv[:, do, :])
            boot_lanes[(2 * do + 1) % 3].dma_start(out=wm_t[:, do, :], in_=wmv[:, do, :])
        w_sb = {"m": wm_t}
        xa_b0 = load_xT(xaT, 0, "xaT", lane0=0, dt=FP8)
        w_sb["v"] = load_wT(wvT, "wv", lane0=1, dt=FP8)
        xb_b0 = load_xT(xbT, 0, "xbT", lane0=0, dt=FP8)
        w_sb["f"] = load_wT(wfT, "wf", lane0=1)

        bfc_sb = consts.tile([P, DIM], BF16)
        nc.sync.dma_start(out=bfc_sb, in_=bfc[:].partition_broadcast(P))

        idn = consts.tile([P, P], BF16)
        make_identity(nc, idn)
        eps_sb = consts.tile([P, 1], F32)
        nc.vector.memset(eps_sb, EPS)

        for b in range(BPC):
            if b == 0:
                x_sb = {"a": xa_b0, "b": xb_b0, "c": xc_b0}
            else:
                x_sb = {
                    "c": load_xT(xcT, b, "xcT", lane0=0, dt=FP8),
                    "a": load_xT(xaT, b, "xaT", lane0=2, dt=FP8),
                    "b": load_xT(xbT, b, "xbT", lane0=1, dt=FP8),
                }
            # per-key exp bias: SCALE*(x_s @ (W_k^T b_q)) - ESHIFT
            ck_sb = xpool.tile([P, 2, SCH], F32, tag="ck")
            nc.gpsimd.dma_start(out=ck_sb, in_=ck[b])

            # --- projections Q^T, K_A^T, K_B^T: [e, s] (e on partitions),
            # stored fp8 (values ~N(0,1), well inside e4m3 range); the softmax
            # 1/sqrt(D) scale is applied later inside the Exp activation ---
            def projT(tag, w_t, x_t, bias_ap=None, dr=False):
                dst = qkv.tile([P, DCH, S], FP8, tag=tag)
                for ec in range(DCH):
                    for sh in range(S // 512):
                        ps = ps512.tile([P, 512], F32, tag="ps512")
                        if dr:
                            for dp in range(DCH // 2):
                                dsl = slice(2 * dp, 2 * dp + 2)
                                nc.tensor.matmul(
                                    ps,
                                    lhsT=w_t[:, dsl, ec * P:(ec + 1) * P],
                                    rhs=x_t[:, dsl, sh * 512:(sh + 1) * 512],
                                    start=(dp == 0),
                                    stop=(dp == DCH // 2 - 1),
                                    perf_mode=mybir.MatmulPerfMode.DoubleRow,
                                )
                        else:
                            for do in range(DCH):
                                nc.tensor.matmul(
                                    ps,
                                    lhsT=w_t[:, do, ec * P:(ec + 1) * P],
                                    rhs=x_t[:, do, sh * 512:(sh + 1) * 512],
                                    start=(do == 0),
                                    stop=(do == DCH - 1),
                                )
                        o = dst[:, ec, sh * 512:(sh + 1) * 512]
                        if bias_ap is not None:
                            nc.scalar.activation(
                                out=o, in_=ps, func=AF.Identity,
                                bias=bias_ap[:, ec:ec + 1], scale=1.0,
                            )
                        else:
                            # alternate the PSUM->SBUF casts across DVE and
                            # ACT (Pool can't read PSUM) so neither engine
                            # gates psum reuse
                            if (ec * 2 + sh) % 2 == 0:
                                nc.vector.tensor_copy(out=o, in_=ps)
                            else:
                                nc.scalar.copy(out=o, in_=ps)
                return dst

            gT_sb = projT("GT", w_sb["m"], x_sb["c"], dr=True)

            # --- V_A, V_B natural layout [s, e] + two ones columns ---
            v_sb = {}
            for name in ("a", "b"):
                dst = qkv.tile([P, SCH, VW], FP8, tag=f"V{name.upper()}")
                nc.vector.memset(dst[:, :, DIM:VW], 1.0)
                for sc in range(SCH):
                    for off, w in ((0, 384), (384, 384)):
                        ps = ps512.tile([P, 512], F32, tag="ps512")
                        for dp in range(DCH // 2):
                            dsl = slice(2 * dp, 2 * dp + 2)
                            nc.tensor.matmul(
                                ps[:, :w],
                                lhsT=x_sb[name][:, dsl, sc * P:(sc + 1) * P],
                                rhs=w_sb["v"][:, dsl, off:off + w],
                                start=(dp == 0),
                                stop=(dp == DCH // 2 - 1),
                                perf_mode=mybir.MatmulPerfMode.DoubleRow,
                            )
                        if (sc + (0 if off else 1)) % 2 == 0:
                            nc.vector.tensor_copy(out=dst[:, sc, off:off + w], in_=ps[:, :w])
                        else:
                            nc.scalar.copy(out=dst[:, sc, off:off + w], in_=ps[:, :w])
                v_sb[name] = dst

            # --- attention + epilogue, per q-block of 512 ---
            for qb in range(S // 512):
                qsl = slice(qb * 512, (qb + 1) * 512)
                # scores^T and exp: e^T[k, q] = exp(x_s[k,:] @ G[q,:] + ck)
                # with x_s^T itself as the stationary operand (no K tiles).
                e_sb = {}
                for si, name in enumerate(("a", "b")):
                    et = epool.tile([P, SCH, 512], FP8, tag=f"e{name.upper()}")
                    for kc in range(SCH):
                        ps = ps512.tile([P, 512], F32, tag="ps512")
                        for ep in range(DCH // 2):
                            esl = slice(2 * ep, 2 * ep + 2)
                            nc.tensor.matmul(
                                ps,
                                lhsT=x_sb[name][:, esl, kc * P:(kc + 1) * P],
                                rhs=gT_sb[:, esl, qsl],
                                start=(ep == 0),
                                stop=(ep == DCH // 2 - 1),
                                perf_mode=mybir.MatmulPerfMode.DoubleRow,
                            )
                        # exp(score/sqrt(D) + SCALE*b_q.W_k.x_s[k] - ESHIFT)
                        nc.scalar.activation(
                            out=et[:, kc, :], in_=ps, func=AF.Exp,
                            bias=ck_sb[:, si, kc:kc + 1], scale=SCALE,
                        )
                    e_sb[name] = et

                # pass 1: interp + layernorm -> z[qi]
                zs = []
                for qi in range(4):
                    qc = qb * 4 + qi
                    qs = slice(qi * P, (qi + 1) * P)

                    xc_t = opool.tile([P, DIM], BF16, tag="xc")
                    nc.gpsimd.dma_start(out=xc_t, in_=xcr[b, qc * P:(qc + 1) * P, :])

                    # interp psums, split 385/385 so every PE pass is longer
                    # than a LDWEIGHTS (135ns); h1 carries the ones columns
                    # -> row-sums at p1 col 383 (= v col 768)
                    pa = {}
                    for name in ("a", "b"):
                        p0 = ps512.tile([P, 512], F32, tag="ps512")
                        p1 = ps512.tile([P, 512], F32, tag="ps512")
                        for kp in range(SCH // 2):
                            ksl = slice(2 * kp, 2 * kp + 2)
                            nc.tensor.matmul(
                                p0[:, 0:385],
                                lhsT=e_sb[name][:, ksl, qs],
                                rhs=v_sb[name][:, ksl, 0:385],
                                start=(kp == 0),
                                stop=(kp == SCH // 2 - 1),
                                perf_mode=mybir.MatmulPerfMode.DoubleRow,
                            )
                        for kp in range(SCH // 2):
                            ksl = slice(2 * kp, 2 * kp + 2)
                            nc.tensor.matmul(
                                p1[:, 0:385],
                                lhsT=e_sb[name][:, ksl, qs],
                                rhs=v_sb[name][:, ksl, 385:VW],
                                start=(kp == 0),
                                stop=(kp == SCH // 2 - 1),
                                perf_mode=mybir.MatmulPerfMode.DoubleRow,
                            )
                        pa[name] = (p0, p1)

                    rcp = {}
                    for name in ("a", "b"):
                        r = spool.tile([P, 1], F32, tag=f"r{name}")
                        nc.vector.reciprocal(r, pa[name][1][:, 383:384])
                        rcp[name] = r

                    # t1 = psA*rA + xc ; t1 += psB*rB   (residual fused)
                    t1 = spool.tile([P, DIM], F32, tag="t1")
                    for (off, w, pi) in ((0, 385, 0), (385, 383, 1)):
                        nc.vector.scalar_tensor_tensor(
                            out=t1[:, off:off + w],
                            in0=pa["a"][pi][:, 0:w],
                            scalar=rcp["a"], in1=xc_t[:, off:off + w],
                            op0=ALU.mult, op1=ALU.add,
                        )
                        nc.vector.scalar_tensor_tensor(
                            out=t1[:, off:off + w],
                            in0=pa["b"][pi][:, 0:w],
                            scalar=rcp["b"], in1=t1[:, off:off + w],
                            op0=ALU.mult, op1=ALU.add,
                        )

                    # layernorm
                    stats = spool.tile([P, 3, 6], F32, tag="st")
                    for g in range(3):
                        nc.vector.bn_stats(
                            out=stats[:, g, :], in_=t1[:, g * 256:(g + 1) * 256]
                        )
                    mv = spool.tile([P, 2], F32, tag="mv")
                    nc.vector.bn_aggr(out=mv, in_=stats)
                    std = spool.tile([P, 1], F32, tag="std")
                    nc.scalar.activation(
                        out=std, in_=mv[:, 1:2], func=AF.Sqrt, bias=eps_sb
                    )
                    rstd = spool.tile([P, 1], F32, tag="rstd")
                    nc.vector.reciprocal(rstd, std)
                    # z = (t1 - mu) * rstd, computed on ACT as
                    # Identity(t1 * rstd + (-mu * rstd)); LayerNorm is now
                    # fully applied BEFORE fc, so the fc copyout is a plain
                    # +b_fc and no mean/std correction is needed after.
                    nmr = spool.tile([P, 1], F32, tag="nmr")
                    nc.vector.tensor_scalar(
                        nmr, mv[:, 0:1], -1.0, rstd, ALU.mult, ALU.mult
                    )
                    # z split DVE/ACT so the transposes (which wait on z)
                    # see ~half the latency
                    z = zpool.tile([P, DIM], BF16, tag="z")
                    nc.vector.tensor_scalar(
                        z[:, 0:256], t1[:, 0:256], rstd, nmr, ALU.mult, ALU.add
                    )
                    nc.scalar.activation(
                        out=z[:, 256:DIM], in_=t1[:, 256:DIM], func=AF.Identity,
                        bias=nmr, scale=rstd,
                    )
                    zs.append(z)

                # pass 2: transpose h + fc + store
                for qi in range(4):
                    qc = qb * 4 + qi
                    z = zs[qi]

                    hT = opool.tile([P, DCH, P], BF16, tag="hT")
                    for ep in range(DCH // 2):
                        pst = pstr.tile([P, 2, P], BF16, tag="pstr")
                        for j in range(2):
                            eo = ep * 2 + j
                            nc.tensor.transpose(
                                pst[:, j], z[:, eo * P:(eo + 1) * P], idn
                            )
                        nc.scalar.copy(out=hT[:, ep * 2:(ep + 1) * 2, :], in_=pst)

                    o_t = opool.tile([P, DIM], BF16, tag="o")
                    for off, w in ((0, 384), (384, 384)):
                        ps = ps512.tile([P, 512], F32, tag="ps512")
                        for eo in range(DCH):
                            nc.tensor.matmul(
                                ps[:, :w],
                                lhsT=hT[:, eo, :],
                                rhs=w_sb["f"][:, eo, off:off + w],
                                start=(eo == 0),
                                stop=(eo == DCH - 1),
                            )
                        nc.vector.scalar_tensor_tensor(
                            out=o_t[:, off:off + w],
                            in0=ps[:, :w], scalar=0.0,
                            in1=bfc_sb[:, off:off + w],
                            op0=ALU.bypass, op1=ALU.add,
                        )
                        # per-chunk store: the last tile's DMA starts after
                        # the first chunk's add instead of after the whole row
                        nc.sync.dma_start(
                            out=out[b, qc * P:(qc + 1) * P, off:off + w],
                            in_=o_t[:, off:off + w],
                        )

    nc.compile()
    return nc


_CACHED_NC = None
_LAST_IN_MAPS = None


def kernel(**inputs) -> np.ndarray:
    global _CACHED_NC, _LAST_IN_MAPS
    bf16 = ml_dtypes.bfloat16
    f32 = np.float32

    xA = np.asarray(inputs["x_A"], dtype=f32)
    xB = np.asarray(inputs["x_B"], dtype=f32)
    xC = np.asarray(inputs["x_C"], dtype=f32)

    fp8 = ml_dtypes.float8_e4m3
    xaT = np.ascontiguousarray(xA.transpose(0, 2, 1)).astype(fp8)
    xbT = np.ascontiguousarray(xB.transpose(0, 2, 1)).astype(fp8)
    xcT = np.ascontiguousarray(xC.transpose(0, 2, 1)).astype(fp8)
    xcr = (xC + 2.0 * np.asarray(inputs["b_v"], dtype=f32)).astype(bf16)

    W_q = np.asarray(inputs["W_q"], dtype=f32)
    W_k = np.asarray(inputs["W_k"], dtype=f32)
    b_q = np.asarray(inputs["b_q"], dtype=f32)
    # scores = Q K^T = x_C (W_q^T W_k) x_s^T + (b_q W_k) x_s^T
    wm = np.ascontiguousarray(W_q.T @ W_k).astype(fp8)
    vk = b_q @ W_k
    # per-key exp bias, laid out [b, p, src, kc] so the DMA is contiguous
    ck_full = np.stack(
        [SCALE * (xA @ vk) - ESHIFT, SCALE * (xB @ vk) - ESHIFT], axis=1
    )  # [B, 2, S]
    ck_arr = np.ascontiguousarray(
        ck_full.reshape(B, 2, S // P, P).transpose(0, 3, 1, 2)
    ).astype(f32)  # [B, P, 2, SCH]
    wvT = np.ascontiguousarray(np.asarray(inputs["W_v"], dtype=f32).T).astype(fp8)

    # fold LayerNorm's gamma/beta into the fc layer (exact):
    #   h = z*gamma + beta;  out = h @ W_fc.T + b_fc
    #     = z @ (W_fc * gamma).T + (b_fc + W_fc @ beta)
    gam = np.asarray(inputs["gamma"], dtype=f32)
    bet = np.asarray(inputs["beta"], dtype=f32)
    W_fc = np.asarray(inputs["W_fc"], dtype=f32)
    wfT = np.ascontiguousarray(W_fc.T * gam[:, None]).astype(bf16)
    bfc = (np.asarray(inputs["b_fc"], dtype=f32) + W_fc @ bet).astype(bf16)

    if _CACHED_NC is None:
        _CACHED_NC = build_bass()
    nc = _CACHED_NC

    in_maps = []
    for c in range(NCORES):
        sl = slice(c * BPC, (c + 1) * BPC)
        in_maps.append({
            "xaT": np.ascontiguousarray(xaT[sl]),
            "xbT": np.ascontiguousarray(xbT[sl]),
            "xcT": np.ascontiguousarray(xcT[sl]),
            "xcr": np.ascontiguousarray(xcr[sl]),
            "wm": wm, "wvT": wvT, "wfT": wfT,
            "ck": np.ascontiguousarray(ck_arr[sl]), "bfc": bfc,
        })

    _LAST_IN_MAPS = in_maps
    res = run_bass_kernel_spmd(nc, in_maps, core_ids=list(range(NCORES)))
    outs = [np.asarray(res.results[i]["out"], dtype=f32) for i in range(NCORES)]
    return np.concatenate(outs, axis=0)


if __name__ == "__main__":
    rng = np.random.default_rng(0)
    fake = {
        "x_A": rng.standard_normal((B, S, DIM), dtype=np.float32),
        "x_B": rng.standard_normal((B, S, DIM), dtype=np.float32),
        "x_C": rng.standard_normal((B, S, DIM), dtype=np.float32),
        "W_q": rng.standard_normal((DIM, DIM), dtype=np.float32) / 27.7,
        "b_q": rng.standard_normal(DIM).astype(np.float32) / 27.7,
        "W_k": rng.standard_normal((DIM, DIM), dtype=np.float32) / 27.7,
        "b_k": rng.standard_normal(DIM).astype(np.float32) / 27.7,
        "W_v": rng.standard_normal((DIM, DIM), dtype=np.float32) / 27.7,
        "b_v": rng.standard_normal(DIM).astype(np.float32) / 27.7,
        "gamma": np.ones(DIM, np.float32),
        "beta": np.zeros(DIM, np.float32),
        "W_fc": rng.standard_normal((DIM, DIM), dtype=np.float32) / 27.7,
        "b_fc": rng.standard_normal(DIM).astype(np.float32) / 27.7,
    }
    o = kernel(**fake)
    print(o.shape, o.dtype)



# revision 66
# speedup vs baseline: 1.1745x; 1.0074x over previous
"""Trainium2 Bass kernel for AdaptiveInterpolationModule (dual-source cross-attention).

Reference computation (B=16, S=1024, D=768):
    Q   = x_C @ W_q.T + b_q
    K_s = x_s @ W_k.T + b_k          (s in {A, B})
    V_s = x_s @ W_v.T + b_v
    attn_s   = softmax(Q K_s^T / sqrt(D))
    interp_s = attn_s V_s
    h   = LayerNorm(interp_A + interp_B + x_C) * gamma + beta
    out = h @ W_fc.T + b_fc

Sharding: data-parallel over batch, 2 batches per core on 8 cores. No collectives.

Math simplifications (exact):
  - scores = Q K_s^T = x_C (W_q^T W_k) x_s^T + (b_q W_k) x_s^T: the Q, K_A and
    K_B projections collapse into ONE projection G^T = (W_q^T W_k)^T x_C^T,
    and the scores matmuls take the already-resident x_s^T tiles as the
    stationary operand directly.
  - b_k never affects the output: scores rows shift by a k-constant -> softmax invariant.
  - b_v contributes exactly +b_v per source (attn rows sum to 1) -> folded into the
    residual input on the host (x_C + 2*b_v).
  - the b_q W_k x_s^T score term is a per-KEY constant: host-computed and fed
    to the Exp activation as a per-partition bias AP (with -ESHIFT folded in);
    the softmax 1/sqrt(D) lives inside the Exp activation's scale.
  - LayerNorm's gamma/beta folded into the fc layer on the host:
    out = h @ (W_fc*gamma).T + (b_fc + W_fc@beta), with h = (t1-mu)*rstd
    applied on the ACT engine while casting to bf16 (scale/bias are
    per-partition APs), so the post-fc copyout is a single +b_fc add.
  - softmax computed without per-row max subtraction: a constant shift ESHIFT keeps
    exp() inside fp8e4m3 range (shift-invariant, exact).

Precision (gate 2e-2):
  - f32: PSUM accumulation, softmax row-sums/normalization, LayerNorm, residual, output.
  - bf16: fc inputs (h and W_fc) -- the accuracy anchor (errors here pass
    straight to the output; attention-path errors average out over keys).
  - fp8e4m3 (+DoubleRow, K=256/matmul -> half the PE passes): all Q/K/V
    projection inputs, Q^T/K^T storage (scores), exp(scores) and V (interp).

Device dataflow per (core, batch):
  - x^T tiles (pre-transposed on host) + W^T tiles (pre-transposed on host).
  - G^T computed transposed [e, s]; V_A, V_B computed natural [s, e]
    with two extra all-ones columns so the interp matmul also produces softmax
    row-sums for free.
  - scores^T[k, q] per q-block of 512: k on partitions, so softmax needs no
    transposes at all; exp PSUM->SBUF fp8 on ACT.
  - interp accumulated over both sources' k-chunks; 1/rowsum applied as per-partition
    scalars during PSUM->SBUF copyout with the residual fused in; LayerNorm
    (bn_stats/bn_aggr); PE-transpose z in bf16 pairs; fc matmul; +b_fc; out.
  - Per q-block the epilogues are emitted in two passes (all interp+LN, then all
    transpose+fc) so the PE's static order never waits on a DVE chain.
  - dma_start issue costs ~600ns of the issuing sequencer, so initial loads are
    split per d-chunk across the sync and gpsimd lanes, critical tensors first.
"""

import sys

import numpy as np

try:
    import concourse.bass as bass
except ImportError:
    sys.path.insert(0, "/opt/trn_rl_repo")
    import concourse.bass as bass

import ml_dtypes
from contextlib import ExitStack

import concourse.mybir as mybir
import concourse.tile as tile
from concourse import bacc
from concourse.bass_utils import run_bass_kernel_spmd
from concourse.masks import make_identity

P = 128
DIM = 768
S = 1024
B = 16
NCORES = 8
BPC = B // NCORES  # batches per core
DCH = DIM // P     # 6 chunks of 128 along D
SCH = S // P       # 8 chunks of 128 along S
EPS = 1e-5
SCALE = 1.0 / float(np.sqrt(DIM))
F32 = mybir.dt.float32
BF16 = mybir.dt.bfloat16
FP8 = mybir.dt.float8e4
# constant softmax shift: exp(score - ESHIFT) keeps values inside fp8e4m3
# range (max score on N(0,1)-scaled data is ~8 -> exp(4.5) = 90 < 448).
# Softmax is shift-invariant so this is exact.
ESHIFT = 3.5

AF = mybir.ActivationFunctionType
ALU = mybir.AluOpType

VW = DIM + 2  # V tile width: 768 value cols + 2 ones cols (row-sum trick)


def build_bass() -> bass.Bass:
    nc = bacc.Bacc()

    xaT = nc.declare_dram_parameter("xaT", [BPC, DIM, S], FP8, isOutput=False)
    xbT = nc.declare_dram_parameter("xbT", [BPC, DIM, S], FP8, isOutput=False)
    xcT = nc.declare_dram_parameter("xcT", [BPC, DIM, S], FP8, isOutput=False)
    xcr = nc.declare_dram_parameter("xcr", [BPC, S, DIM], BF16, isOutput=False)
    # wm = W_q^T @ W_k: scores = x_C @ wm @ x_s^T, so Q/K_A/K_B projections
    # collapse into ONE projection G^T = wm^T x_C^T; the b_q @ W_k @ x_s^T
    # score term is precomputed on the host as a per-key exp bias (ck).
    wm = nc.declare_dram_parameter("wm", [DIM, DIM], FP8, isOutput=False)
    wvT = nc.declare_dram_parameter("wvT", [DIM, DIM], FP8, isOutput=False)
    wfT = nc.declare_dram_parameter("wfT", [DIM, DIM], BF16, isOutput=False)
    ck = nc.declare_dram_parameter("ck", [BPC, P, 2, SCH], F32, isOutput=False)
    bfc = nc.declare_dram_parameter("bfc", [DIM], BF16, isOutput=False)
    out = nc.declare_dram_parameter("out", [BPC, S, DIM], BF16, isOutput=True)

    with tile.TileContext(nc) as tc, ExitStack() as ctx:
        consts = ctx.enter_context(tc.tile_pool(name="consts", bufs=1))
        wpool = ctx.enter_context(tc.tile_pool(name="wpool", bufs=1))
        xpool = ctx.enter_context(tc.tile_pool(name="xpool", bufs=1))
        qkv = ctx.enter_context(tc.tile_pool(name="qkv", bufs=1))
        epool = ctx.enter_context(tc.tile_pool(name="epool", bufs=3))
        spool = ctx.enter_context(tc.tile_pool(name="spool", bufs=4))
        zpool = ctx.enter_context(tc.tile_pool(name="zpool", bufs=5))
        opool = ctx.enter_context(tc.tile_pool(name="opool", bufs=4))
        # one bank-sized psum pool: 6 of the 8 banks rotate through all
        # matmul outputs (projections, scores, interp 385-slices, fc); the
        # remaining 2 banks hold the transpose pairs.
        ps512 = ctx.enter_context(tc.tile_pool(name="ps512", bufs=6, space="PSUM"))
        pstr = ctx.enter_context(tc.tile_pool(name="pstr", bufs=2, space="PSUM"))

        # --- first-needed DMAs first: the G projection needs xcT + wm.
        # dma_start issue costs ~600ns of the issuing engine's sequencer, so
        # spread the initial loads across otherwise-idle sequencers; the very
        # first tensors (xc, wm) go 3-wide so the PE can start sooner.
        dma_lanes = [nc.sync, nc.gpsimd]
        boot_lanes = [nc.sync, nc.gpsimd, nc.scalar]

        def load_xT(h, b, tag, lane0=0, dt=BF16, lanes=None):
            lanes = lanes or dma_lanes
            t = xpool.tile([P, DCH, S], dt, tag=tag)
            v = h[b].rearrange("(o p) s -> p o s", p=P)
            for do in range(DCH):
                eng = lanes[(lane0 + do) % len(lanes)]
                eng.dma_start(out=t[:, do, :], in_=v[:, do, :])
            return t

        def load_wT(h, tag, lane0=0, dt=BF16, lanes=None):
            lanes = lanes or dma_lanes
            t = wpool.tile([P, DCH, DIM], dt, tag=tag)
            v = h[:].rearrange("(o p) e -> p o e", p=P)
            for do in range(DCH):
                eng = lanes[(lane0 + do) % len(lanes)]
                eng.dma_start(out=t[:, do, :], in_=v[:, do, :])
            return t

        # The G projection's inputs gate the first matmul: interleave the
        # xc/wm chunk DMAs 3 queues wide so the first DR pair (d0,d1 of both)
        # lands after ~4 transfers instead of after all 12.
        xc_b0 = xpool.tile([P, DCH, S], FP8, tag="xcT")
        wm_t = wpool.tile([P, DCH, DIM], FP8, tag="wm")
        xcv = xcT[0].rearrange("(o p) s -> p o s", p=P)
        wmv = wm[:].rearrange("(o p) e -> p o e", p=P)
        for do in range(DCH):
            boot_lanes[(2 * do) % 3].dma_start(out=xc_b0[:, do, :], in_=xcv[:, do, :])
            boot_lanes[(2 * do + 1) % 3].dma_start(out=wm_t[:, do, :], in_=wmv[:, do, :])
        w_sb = {"m": wm_t}
        xa_b0 = load_xT(xaT, 0, "xaT", lane0=0, dt=FP8)
        w_sb["v"] = load_wT(wvT, "wv", lane0=1, dt=FP8)
        xb_b0 = load_xT(xbT, 0, "xbT", lane0=0, dt=FP8)
        w_sb["f"] = load_wT(wfT, "wf", lane0=1)

        bfc_sb = consts.tile([P, DIM], BF16)
        nc.sync.dma_start(out=bfc_sb, in_=bfc[:].partition_broadcast(P))

        idn = consts.tile([P, P], BF16)
        make_identity(nc, idn)
        eps_sb = consts.tile([P, 1], F32)
        nc.vector.memset(eps_sb, EPS)

        for b in range(BPC):
            if b == 0:
                x_sb = {"a": xa_b0, "b": xb_b0, "c": xc_b0}
            else:
                x_sb = {
                    "c": load_xT(xcT, b, "xcT", lane0=0, dt=FP8),
                    "a": load_xT(xaT, b, "xaT", lane0=2, dt=FP8),
                    "b": load_xT(xbT, b, "xbT", lane0=1, dt=FP8),
                }
            # per-key exp bias: SCALE*(x_s @ (W_k^T b_q)) - ESHIFT
            ck_sb = xpool.tile([P, 2, SCH], F32, tag="ck")
            nc.gpsimd.dma_start(out=ck_sb, in_=ck[b])

            # --- projections Q^T, K_A^T, K_B^T: [e, s] (e on partitions),
            # stored fp8 (values ~N(0,1), well inside e4m3 range); the softmax
            # 1/sqrt(D) scale is applied later inside the Exp activation ---
            def projT(tag, w_t, x_t, bias_ap=None, dr=False):
                dst = qkv.tile([P, DCH, S], FP8, tag=tag)
                for ec in range(DCH):
                    for sh in range(S // 512):
                        ps = ps512.tile([P, 512], F32, tag="ps512")
                        if dr:
                            for dp in range(DCH // 2):
                                dsl = slice(2 * dp, 2 * dp + 2)
                                nc.tensor.matmul(
                                    ps,
                                    lhsT=w_t[:, dsl, ec * P:(ec + 1) * P],
                                    rhs=x_t[:, dsl, sh * 512:(sh + 1) * 512],
                                    start=(dp == 0),
                                    stop=(dp == DCH // 2 - 1),
                                    perf_mode=mybir.MatmulPerfMode.DoubleRow,
                                )
                        else:
                            for do in range(DCH):
                                nc.tensor.matmul(
                                    ps,
                                    lhsT=w_t[:, do, ec * P:(ec + 1) * P],
                                    rhs=x_t[:, do, sh * 512:(sh + 1) * 512],
                                    start=(do == 0),
                                    stop=(do == DCH - 1),
                                )
                        o = dst[:, ec, sh * 512:(sh + 1) * 512]
                        if bias_ap is not None:
                            nc.scalar.activation(
                                out=o, in_=ps, func=AF.Identity,
                                bias=bias_ap[:, ec:ec + 1], scale=1.0,
                            )
                        else:
                            # alternate the PSUM->SBUF casts across DVE and
                            # ACT (Pool can't read PSUM) so neither engine
                            # gates psum reuse
                            if (ec * 2 + sh) % 2 == 0:
                                nc.vector.tensor_copy(out=o, in_=ps)
                            else:
                                nc.scalar.copy(out=o, in_=ps)
                return dst

            gT_sb = projT("GT", w_sb["m"], x_sb["c"], dr=True)

            # --- V_A, V_B natural layout [s, e] + two ones columns ---
            v_sb = {}
            for name in ("a", "b"):
                dst = qkv.tile([P, SCH, VW], FP8, tag=f"V{name.upper()}")
                nc.vector.memset(dst[:, :, DIM:VW], 1.0)
                for sc in range(SCH):
                    for off, w in ((0, 384), (384, 384)):
                        ps = ps512.tile([P, 512], F32, tag="ps512")
                        for dp in range(DCH // 2):
                            dsl = slice(2 * dp, 2 * dp + 2)
                            nc.tensor.matmul(
                                ps[:, :w],
                                lhsT=x_sb[name][:, dsl, sc * P:(sc + 1) * P],
                                rhs=w_sb["v"][:, dsl, off:off + w],
                                start=(dp == 0),
                                stop=(dp == DCH // 2 - 1),
                                perf_mode=mybir.MatmulPerfMode.DoubleRow,
                            )
                        if (sc + (0 if off else 1)) % 2 == 0:
                            nc.vector.tensor_copy(out=dst[:, sc, off:off + w], in_=ps[:, :w])
                        else:
                            nc.scalar.copy(out=dst[:, sc, off:off + w], in_=ps[:, :w])
                v_sb[name] = dst

            # --- attention + epilogue, per q-block of 512 ---
            for qb in range(S // 512):
                qsl = slice(qb * 512, (qb + 1) * 512)
                # scores^T and exp: e^T[k, q] = exp(x_s[k,:] @ G[q,:] + ck)
                # with x_s^T itself as the stationary operand (no K tiles).
                e_sb = {}
                for si, name in enumerate(("a", "b")):
                    et = epool.tile([P, SCH, 512], FP8, tag=f"e{name.upper()}")
                    for kc in range(SCH):
                        ps = ps512.tile([P, 512], F32, tag="ps512")
                        for ep in range(DCH // 2):
                            esl = slice(2 * ep, 2 * ep + 2)
                            nc.tensor.matmul(
                                ps,
                                lhsT=x_sb[name][:, esl, kc * P:(kc + 1) * P],
                                rhs=gT_sb[:, esl, qsl],
                                start=(ep == 0),
                                stop=(ep == DCH // 2 - 1),
                                perf_mode=mybir.MatmulPerfMode.DoubleRow,
                            )
                        # exp(score/sqrt(D) + SCALE*b_q.W_k.x_s[k] - ESHIFT)
                        nc.scalar.activation(
                            out=et[:, kc, :], in_=ps, func=AF.Exp,
                            bias=ck_sb[:, si, kc:kc + 1], scale=SCALE,
                        )
                    e_sb[name] = et

                # pass 1: interp + layernorm -> z[qi]
                zs = []
                for qi in range(4):
                    qc = qb * 4 + qi
                    qs = slice(qi * P, (qi + 1) * P)

                    xc_t = opool.tile([P, DIM], BF16, tag="xc")
                    nc.gpsimd.dma_start(out=xc_t, in_=xcr[b, qc * P:(qc + 1) * P, :])

                    # interp psums, split 385/385 so every PE pass is longer
                    # than a LDWEIGHTS (135ns); h1 carries the ones columns
                    # -> row-sums at p1 col 383 (= v col 768)
                    pa = {}
                    for name in ("a", "b"):
                        p0 = ps512.tile([P, 512], F32, tag="ps512")
                        p1 = ps512.tile([P, 512], F32, tag="ps512")
                        for kp in range(SCH // 2):
                            ksl = slice(2 * kp, 2 * kp + 2)
                            nc.tensor.matmul(
                                p0[:, 0:385],
                                lhsT=e_sb[name][:, ksl, qs],
                                rhs=v_sb[name][:, ksl, 0:385],
                                start=(kp == 0),
                                stop=(kp == SCH // 2 - 1),
                                perf_mode=mybir.MatmulPerfMode.DoubleRow,
                            )
                        for kp in range(SCH // 2):
                            ksl = slice(2 * kp, 2 * kp + 2)
                            nc.tensor.matmul(
                                p1[:, 0:385],
                                lhsT=e_sb[name][:, ksl, qs],
                                rhs=v_sb[name][:, ksl, 385:VW],
                                start=(kp == 0),
                                stop=(kp == SCH // 2 - 1),
                                perf_mode=mybir.MatmulPerfMode.DoubleRow,
                            )
                        pa[name] = (p0, p1)

                    rcp = {}
                    for name in ("a", "b"):
                        r = spool.tile([P, 1], F32, tag=f"r{name}")
                        nc.vector.reciprocal(r, pa[name][1][:, 383:384])
                        rcp[name] = r

                    # t1 = psA*rA + xc ; t1 += psB*rB   (residual fused)
                    t1 = spool.tile([P, DIM], F32, tag="t1")
                    for (off, w, pi) in ((0, 385, 0), (385, 383, 1)):
                        nc.vector.scalar_tensor_tensor(
                            out=t1[:, off:off + w],
                            in0=pa["a"][pi][:, 0:w],
                            scalar=rcp["a"], in1=xc_t[:, off:off + w],
                            op0=ALU.mult, op1=ALU.add,
                        )
                        nc.vector.scalar_tensor_tensor(
                            out=t1[:, off:off + w],
                            in0=pa["b"][pi][:, 0:w],
                            scalar=rcp["b"], in1=t1[:, off:off + w],
                            op0=ALU.mult, op1=ALU.add,
                        )

                    # layernorm
                    stats = spool.tile([P, 3, 6], F32, tag="st")
                    for g in range(3):
                        nc.vector.bn_stats(
                            out=stats[:, g, :], in_=t1[:, g * 256:(g + 1) * 256]
                        )
                    mv = spool.tile([P, 2], F32, tag="mv")
                    nc.vector.bn_aggr(out=mv, in_=stats)
                    std = spool.tile([P, 1], F32, tag="std")
                    nc.scalar.activation(
                        out=std, in_=mv[:, 1:2], func=AF.Sqrt, bias=eps_sb
                    )
                    rstd = spool.tile([P, 1], F32, tag="rstd")
                    nc.vector.reciprocal(rstd, std)
                    # z = (t1 - mu) * rstd, computed on ACT as
                    # Identity(t1 * rstd + (-mu * rstd)); LayerNorm is now
                    # fully applied BEFORE fc, so the fc copyout is a plain
                    # +b_fc and no mean/std correction is needed after.
                    nmr = spool.tile([P, 1], F32, tag="nmr")
                    nc.vector.tensor_scalar(
                        nmr, mv[:, 0:1], -1.0, rstd, ALU.mult, ALU.mult
                    )
                    # z split DVE/ACT so the transposes (which wait on z)
                    # see ~half the latency
                    z = zpool.tile([P, DIM], BF16, tag="z")
                    nc.vector.tensor_scalar(
                        z[:, 0:256], t1[:, 0:256], rstd, nmr, ALU.mult, ALU.add
                    )
                    nc.scalar.activation(
                        out=z[:, 256:DIM], in_=t1[:, 256:DIM], func=AF.Identity,
                        bias=nmr, scale=rstd,
                    )
                    zs.append(z)

                # pass 2: transpose h + fc + store
                for qi in range(4):
                    qc = qb * 4 + qi
                    z = zs[qi]

                    hT = opool.tile([P, DCH, P], BF16, tag="hT")
                    for ep in range(DCH // 2):
                        pst = pstr.tile([P, 2, P], BF16, tag="pstr")
                        for j in range(2):
                            eo = ep * 2 + j
                            nc.tensor.transpose(
                                pst[:, j], z[:, eo * P:(eo + 1) * P], idn
                            )
                        nc.scalar.copy(out=hT[:, ep * 2:(ep + 1) * 2, :], in_=pst)

                    o_t = opool.tile([P, DIM], BF16, tag="o")
                    for off, w in ((0, 384), (384, 384)):
                        ps = ps512.tile([P, 512], F32, tag="ps512")
                        for eo in range(DCH):
                            nc.tensor.matmul(
                                ps[:, :w],
                                lhsT=hT[:, eo, :],
                                rhs=w_sb["f"][:, eo, off:off + w],
                                start=(eo == 0),
                                stop=(eo == DCH - 1),
                            )
                        nc.vector.scalar_tensor_tensor(
                            out=o_t[:, off:off + w],
                            in0=ps[:, :w], scalar=0.0,
                            in1=bfc_sb[:, off:off + w],
                            op0=ALU.bypass, op1=ALU.add,
                        )
                        # per-chunk store: the last tile's DMA starts after
                        # the first chunk's add instead of after the whole row
                        nc.sync.dma_start(
                            out=out[b, qc * P:(qc + 1) * P, off:off + w],
                            in_=o_t[:, off:off + w],
                        )

    nc.compile()
    return nc


_CACHED_NC = None
_LAST_IN_MAPS = None


def kernel(**inputs) -> np.ndarray:
    global _CACHED_NC, _LAST_IN_MAPS
    bf16 = ml_dtypes.bfloat16
    f32 = np.float32

    xA = np.asarray(inputs["x_A"], dtype=f32)
    xB = np.asarray(inputs["x_B"], dtype=f32)
    xC = np.asarray(inputs["x_C"], dtype=f32)

    fp8 = ml_dtypes.float8_e4m3
    xaT = np.ascontiguousarray(xA.transpose(0, 2, 1)).astype(fp8)
    xbT = np.ascontiguousarray(xB.transpose(0, 2, 1)).astype(fp8)
    xcT = np.ascontiguousarray(xC.transpose(0, 2, 1)).astype(fp8)
    xcr = (xC + 2.0 * np.asarray(inputs["b_v"], dtype=f32)).astype(bf16)

    W_q = np.asarray(inputs["W_q"], dtype=f32)
    W_k = np.asarray(inputs["W_k"], dtype=f32)
    b_q = np.asarray(inputs["b_q"], dtype=f32)
    # scores = Q K^T = x_C (W_q^T W_k) x_s^T + (b_q W_k) x_s^T
    wm = np.ascontiguousarray(W_q.T @ W_k).astype(fp8)
    vk = b_q @ W_k
    # per-key exp bias, laid out [b, p, src, kc] so the DMA is contiguous
    ck_full = np.stack(
        [SCALE * (xA @ vk) - ESHIFT, SCALE * (xB @ vk) - ESHIFT], axis=1
    )  # [B, 2, S]
    ck_arr = np.ascontiguousarray(
        ck_full.reshape(B, 2, S // P, P).transpose(0, 3, 1, 2)
    ).astype(f32)  # [B, P, 2, SCH]
    wvT = np.ascontiguousarray(np.asarray(inputs["W_v"], dtype=f32).T).astype(fp8)

    # fold LayerNorm's gamma/beta into the fc layer (exact):
    #   h = z*gamma + beta;  out = h @ W_fc.T + b_fc
    #     = z @ (W_fc * gamma).T + (b_fc + W_fc @ beta)
    gam = np.asarray(inputs["gamma"], dtype=f32)
    bet = np.asarray(inputs["beta"], dtype=f32)
    W_fc = np.asarray(inputs["W_fc"], dtype=f32)
    wfT = np.ascontiguousarray(W_fc.T * gam[:, None]).astype(bf16)
    bfc = (np.asarray(inputs["b_fc"], dtype=f32) + W_fc @ bet).astype(bf16)

    if _CACHED_NC is None:
        _CACHED_NC = build_bass()
    nc = _CACHED_NC

    in_maps = []
    for c in range(NCORES):
        sl = slice(c * BPC, (c + 1) * BPC)
        in_maps.append({
            "xaT": np.ascontiguousarray(xaT[sl]),
            "xbT": np.ascontiguousarray(xbT[sl]),
            "xcT": np.ascontiguousarray(xcT[sl]),
            "xcr": np.ascontiguousarray(xcr[sl]),
            "wm": wm, "wvT": wvT, "wfT": wfT,
            "ck": np.ascontiguousarray(ck_arr[sl]), "bfc": bfc,
        })

    _LAST_IN_MAPS = in_maps
    res = run_bass_kernel_spmd(nc, in_maps, core_ids=list(range(NCORES)))
    outs = [np.asarray(res.results[i]["out"], dtype=f32) for i in range(NCORES)]
    return np.concatenate(outs, axis=0)


if __name__ == "__main__":
    rng = np.random.default_rng(0)
    fake = {
        "x_A": rng.standard_normal((B, S, DIM), dtype=np.float32),
        "x_B": rng.standard_normal((B, S, DIM), dtype=np.float32),
        "x_C": rng.standard_normal((B, S, DIM), dtype=np.float32),
        "W_q": rng.standard_normal((DIM, DIM), dtype=np.float32) / 27.7,
        "b_q": rng.standard_normal(DIM).astype(np.float32) / 27.7,
        "W_k": rng.standard_normal((DIM, DIM), dtype=np.float32) / 27.7,
        "b_k": rng.standard_normal(DIM).astype(np.float32) / 27.7,
        "W_v": rng.standard_normal((DIM, DIM), dtype=np.float32) / 27.7,
        "b_v": rng.standard_normal(DIM).astype(np.float32) / 27.7,
        "gamma": np.ones(DIM, np.float32),
        "beta": np.zeros(DIM, np.float32),
        "W_fc": rng.standard_normal((DIM, DIM), dtype=np.float32) / 27.7,
        "b_fc": rng.standard_normal(DIM).astype(np.float32) / 27.7,
    }
    o = kernel(**fake)
    print(o.shape, o.dtype)



# revision 70
# speedup vs baseline: 1.1787x; 1.0036x over previous
"""Trainium2 Bass kernel for AdaptiveInterpolationModule (dual-source cross-attention).

Reference computation (B=16, S=1024, D=768):
    Q   = x_C @ W_q.T + b_q
    K_s = x_s @ W_k.T + b_k          (s in {A, B})
    V_s = x_s @ W_v.T + b_v
    attn_s   = softmax(Q K_s^T / sqrt(D))
    interp_s = attn_s V_s
    h   = LayerNorm(interp_A + interp_B + x_C) * gamma + beta
    out = h @ W_fc.T + b_fc

Sharding: data-parallel over batch, 2 batches per core on 8 cores. No collectives.

Math simplifications (exact):
  - scores = Q K_s^T = x_C (W_q^T W_k) x_s^T + (b_q W_k) x_s^T: the Q, K_A and
    K_B projections collapse into ONE projection G^T = (W_q^T W_k)^T x_C^T,
    and the scores matmuls take the already-resident x_s^T tiles as the
    stationary operand directly.
  - b_k never affects the output: scores rows shift by a k-constant -> softmax invariant.
  - b_v contributes exactly +b_v per source (attn rows sum to 1) -> folded into the
    residual input on the host (x_C + 2*b_v).
  - the b_q W_k x_s^T score term is a per-KEY constant: host-computed and fed
    to the Exp activation as a per-partition bias AP (with -ESHIFT folded in);
    the softmax 1/sqrt(D) lives inside the Exp activation's scale.
  - LayerNorm's gamma/beta folded into the fc layer on the host:
    out = h @ (W_fc*gamma).T + (b_fc + W_fc@beta), with h = (t1-mu)*rstd
    applied on the ACT engine while casting to bf16 (scale/bias are
    per-partition APs), so the post-fc copyout is a single +b_fc add.
  - softmax computed without per-row max subtraction: a constant shift ESHIFT keeps
    exp() inside fp8e4m3 range (shift-invariant, exact).

Precision (gate 2e-2):
  - f32: PSUM accumulation, softmax row-sums/normalization, LayerNorm, residual, output.
  - bf16: fc inputs (h and W_fc) -- the accuracy anchor (errors here pass
    straight to the output; attention-path errors average out over keys).
  - fp8e4m3 (+DoubleRow, K=256/matmul -> half the PE passes): all Q/K/V
    projection inputs, Q^T/K^T storage (scores), exp(scores) and V (interp).

Device dataflow per (core, batch):
  - x^T tiles (pre-transposed on host) + W^T tiles (pre-transposed on host).
  - G^T computed transposed [e, s]; V_A, V_B computed natural [s, e]
    with two extra all-ones columns so the interp matmul also produces softmax
    row-sums for free.
  - scores^T[k, q] per q-block of 512: k on partitions, so softmax needs no
    transposes at all; exp PSUM->SBUF fp8 on ACT.
  - interp accumulated over both sources' k-chunks; 1/rowsum applied as per-partition
    scalars during PSUM->SBUF copyout with the residual fused in; LayerNorm
    (bn_stats/bn_aggr); PE-transpose z in bf16 pairs; fc matmul; +b_fc; out.
  - Per q-block the epilogues are emitted in two passes (all interp+LN, then all
    transpose+fc) so the PE's static order never waits on a DVE chain.
  - dma_start issue costs ~600ns of the issuing sequencer, so initial loads are
    split per d-chunk across the sync and gpsimd lanes, critical tensors first.
"""

import sys

import numpy as np

try:
    import concourse.bass as bass
except ImportError:
    sys.path.insert(0, "/opt/trn_rl_repo")
    import concourse.bass as bass

import ml_dtypes
from contextlib import ExitStack

import concourse.mybir as mybir
import concourse.tile as tile
from concourse import bacc
from concourse.bass_utils import run_bass_kernel_spmd
from concourse.masks import make_identity

P = 128
DIM = 768
S = 1024
B = 16
NCORES = 8
BPC = B // NCORES  # batches per core
DCH = DIM // P     # 6 chunks of 128 along D
SCH = S // P       # 8 chunks of 128 along S
EPS = 1e-5
SCALE = 1.0 / float(np.sqrt(DIM))
F32 = mybir.dt.float32
BF16 = mybir.dt.bfloat16
FP8 = mybir.dt.float8e4
# constant softmax shift: exp(score - ESHIFT) keeps values inside fp8e4m3
# range (max score on N(0,1)-scaled data is ~8 -> exp(4.5) = 90 < 448).
# Softmax is shift-invariant so this is exact.
ESHIFT = 3.5

AF = mybir.ActivationFunctionType
ALU = mybir.AluOpType

VW = DIM + 2  # V tile width: 768 value cols + 2 ones cols (row-sum trick)


def build_bass() -> bass.Bass:
    nc = bacc.Bacc()

    xaT = nc.declare_dram_parameter("xaT", [BPC, DIM, S], FP8, isOutput=False)
    xbT = nc.declare_dram_parameter("xbT", [BPC, DIM, S], FP8, isOutput=False)
    xcT = nc.declare_dram_parameter("xcT", [BPC, DIM, S], FP8, isOutput=False)
    xcr = nc.declare_dram_parameter("xcr", [BPC, S, DIM], BF16, isOutput=False)
    # wm = W_q^T @ W_k: scores = x_C @ wm @ x_s^T, so Q/K_A/K_B projections
    # collapse into ONE projection G^T = wm^T x_C^T; the b_q @ W_k @ x_s^T
    # score term is precomputed on the host as a per-key exp bias (ck).
    wm = nc.declare_dram_parameter("wm", [DIM, DIM], FP8, isOutput=False)
    wvT = nc.declare_dram_parameter("wvT", [DIM, DIM], FP8, isOutput=False)
    wfT = nc.declare_dram_parameter("wfT", [DIM, DIM], BF16, isOutput=False)
    ck = nc.declare_dram_parameter("ck", [BPC, P, 2, SCH], F32, isOutput=False)
    bfc = nc.declare_dram_parameter("bfc", [DIM], BF16, isOutput=False)
    out = nc.declare_dram_parameter("out", [BPC, S, DIM], BF16, isOutput=True)

    with tile.TileContext(nc) as tc, ExitStack() as ctx:
        consts = ctx.enter_context(tc.tile_pool(name="consts", bufs=1))
        wpool = ctx.enter_context(tc.tile_pool(name="wpool", bufs=1))
        xpool = ctx.enter_context(tc.tile_pool(name="xpool", bufs=1))
        qkv = ctx.enter_context(tc.tile_pool(name="qkv", bufs=1))
        epool = ctx.enter_context(tc.tile_pool(name="epool", bufs=3))
        spool = ctx.enter_context(tc.tile_pool(name="spool", bufs=4))
        zpool = ctx.enter_context(tc.tile_pool(name="zpool", bufs=6))
        opool = ctx.enter_context(tc.tile_pool(name="opool", bufs=4))
        # one bank-sized psum pool: 6 of the 8 banks rotate through all
        # matmul outputs (projections, scores, interp 385-slices, fc); the
        # remaining 2 banks hold the transpose pairs.
        ps512 = ctx.enter_context(tc.tile_pool(name="ps512", bufs=6, space="PSUM"))
        pstr = ctx.enter_context(tc.tile_pool(name="pstr", bufs=2, space="PSUM"))

        # --- first-needed DMAs first: the G projection needs xcT + wm.
        # dma_start issue costs ~600ns of the issuing engine's sequencer, so
        # spread the initial loads across otherwise-idle sequencers; the very
        # first tensors (xc, wm) go 3-wide so the PE can start sooner.
        dma_lanes = [nc.sync, nc.gpsimd]
        boot_lanes = [nc.sync, nc.gpsimd, nc.scalar]

        def load_xT(h, b, tag, lane0=0, dt=BF16, lanes=None):
            lanes = lanes or dma_lanes
            t = xpool.tile([P, DCH, S], dt, tag=tag)
            v = h[b].rearrange("(o p) s -> p o s", p=P)
            for do in range(DCH):
                eng = lanes[(lane0 + do) % len(lanes)]
                eng.dma_start(out=t[:, do, :], in_=v[:, do, :])
            return t

        def load_wT(h, tag, lane0=0, dt=BF16, lanes=None):
            lanes = lanes or dma_lanes
            t = wpool.tile([P, DCH, DIM], dt, tag=tag)
            v = h[:].rearrange("(o p) e -> p o e", p=P)
            for do in range(DCH):
                eng = lanes[(lane0 + do) % len(lanes)]
                eng.dma_start(out=t[:, do, :], in_=v[:, do, :])
            return t

        # The G projection's inputs gate the first matmul: interleave the
        # xc/wm chunk DMAs 3 queues wide so the first DR pair (d0,d1 of both)
        # lands after ~4 transfers instead of after all 12.
        xc_b0 = xpool.tile([P, DCH, S], FP8, tag="xcT")
        wm_t = wpool.tile([P, DCH, DIM], FP8, tag="wm")
        xcv = xcT[0].rearrange("(o p) s -> p o s", p=P)
        wmv = wm[:].rearrange("(o p) e -> p o e", p=P)
        for do in range(DCH):
            boot_lanes[(2 * do) % 3].dma_start(out=xc_b0[:, do, :], in_=xcv[:, do, :])
            boot_lanes[(2 * do + 1) % 3].dma_start(out=wm_t[:, do, :], in_=wmv[:, do, :])
        w_sb = {"m": wm_t}
        xa_b0 = load_xT(xaT, 0, "xaT", lane0=0, dt=FP8)
        w_sb["v"] = load_wT(wvT, "wv", lane0=1, dt=FP8)
        xb_b0 = load_xT(xbT, 0, "xbT", lane0=0, dt=FP8)
        w_sb["f"] = load_wT(wfT, "wf", lane0=1)

        bfc_sb = consts.tile([P, DIM], BF16)
        nc.sync.dma_start(out=bfc_sb, in_=bfc[:].partition_broadcast(P))

        idn = consts.tile([P, P], BF16)
        make_identity(nc, idn)
        eps_sb = consts.tile([P, 1], F32)
        nc.vector.memset(eps_sb, EPS)

        # stage 2 (transpose z + fc + store) for one 128-row tile. Emitted
        # one full q-block BEHIND stage 1 (see the pending queue below), so
        # the transposes never wait on the z chain (~5.7us latency) and the
        # PE work between interp groups gives the DVE time to drain psums.
        def stage2(sb, qc, z):
            hT = opool.tile([P, DCH, P], BF16, tag="hT")
            for ep in range(DCH // 2):
                pst = pstr.tile([P, 2, P], BF16, tag="pstr")
                for j in range(2):
                    eo = ep * 2 + j
                    nc.tensor.transpose(
                        pst[:, j], z[:, eo * P:(eo + 1) * P], idn
                    )
                nc.scalar.copy(out=hT[:, ep * 2:(ep + 1) * 2, :], in_=pst)

            o_t = opool.tile([P, DIM], BF16, tag="o")
            for off, w in ((0, 384), (384, 384)):
                ps = ps512.tile([P, 512], F32, tag="ps512")
                for eo in range(DCH):
                    nc.tensor.matmul(
                        ps[:, :w],
                        lhsT=hT[:, eo, :],
                        rhs=w_sb["f"][:, eo, off:off + w],
                        start=(eo == 0),
                        stop=(eo == DCH - 1),
                    )
                nc.vector.scalar_tensor_tensor(
                    out=o_t[:, off:off + w],
                    in0=ps[:, :w], scalar=0.0,
                    in1=bfc_sb[:, off:off + w],
                    op0=ALU.bypass, op1=ALU.add,
                )
                nc.sync.dma_start(
                    out=out[sb, qc * P:(qc + 1) * P, off:off + w],
                    in_=o_t[:, off:off + w],
                )

        pending = []

        for b in range(BPC):
            if b == 0:
                x_sb = {"a": xa_b0, "b": xb_b0, "c": xc_b0}
            else:
                x_sb = {
                    "c": load_xT(xcT, b, "xcT", lane0=0, dt=FP8),
                    "a": load_xT(xaT, b, "xaT", lane0=2, dt=FP8),
                    "b": load_xT(xbT, b, "xbT", lane0=1, dt=FP8),
                }
            # per-key exp bias: SCALE*(x_s @ (W_k^T b_q)) - ESHIFT
            ck_sb = xpool.tile([P, 2, SCH], F32, tag="ck")
            nc.gpsimd.dma_start(out=ck_sb, in_=ck[b])

            # --- projections Q^T, K_A^T, K_B^T: [e, s] (e on partitions),
            # stored fp8 (values ~N(0,1), well inside e4m3 range); the softmax
            # 1/sqrt(D) scale is applied later inside the Exp activation ---
            def projT(tag, w_t, x_t, bias_ap=None, dr=False):
                dst = qkv.tile([P, DCH, S], FP8, tag=tag)
                for ec in range(DCH):
                    for sh in range(S // 512):
                        ps = ps512.tile([P, 512], F32, tag="ps512")
                        if dr:
                            for dp in range(DCH // 2):
                                dsl = slice(2 * dp, 2 * dp + 2)
                                nc.tensor.matmul(
                                    ps,
                                    lhsT=w_t[:, dsl, ec * P:(ec + 1) * P],
                                    rhs=x_t[:, dsl, sh * 512:(sh + 1) * 512],
                                    start=(dp == 0),
                                    stop=(dp == DCH // 2 - 1),
                                    perf_mode=mybir.MatmulPerfMode.DoubleRow,
                                )
                        else:
                            for do in range(DCH):
                                nc.tensor.matmul(
                                    ps,
                                    lhsT=w_t[:, do, ec * P:(ec + 1) * P],
                                    rhs=x_t[:, do, sh * 512:(sh + 1) * 512],
                                    start=(do == 0),
                                    stop=(do == DCH - 1),
                                )
                        o = dst[:, ec, sh * 512:(sh + 1) * 512]
                        if bias_ap is not None:
                            nc.scalar.activation(
                                out=o, in_=ps, func=AF.Identity,
                                bias=bias_ap[:, ec:ec + 1], scale=1.0,
                            )
                        else:
                            # alternate the PSUM->SBUF casts across DVE and
                            # ACT (Pool can't read PSUM) so neither engine
                            # gates psum reuse
                            if (ec * 2 + sh) % 2 == 0:
                                nc.vector.tensor_copy(out=o, in_=ps)
                            else:
                                nc.scalar.copy(out=o, in_=ps)
                return dst

            gT_sb = projT("GT", w_sb["m"], x_sb["c"], dr=True)

            # --- V_A, V_B natural layout [s, e] + two ones columns ---
            v_sb = {}
            for name in ("a", "b"):
                dst = qkv.tile([P, SCH, VW], FP8, tag=f"V{name.upper()}")
                nc.vector.memset(dst[:, :, DIM:VW], 1.0)
                for sc in range(SCH):
                    for off, w in ((0, 384), (384, 384)):
                        ps = ps512.tile([P, 512], F32, tag="ps512")
                        for dp in range(DCH // 2):
                            dsl = slice(2 * dp, 2 * dp + 2)
                            nc.tensor.matmul(
                                ps[:, :w],
                                lhsT=x_sb[name][:, dsl, sc * P:(sc + 1) * P],
                                rhs=w_sb["v"][:, dsl, off:off + w],
                                start=(dp == 0),
                                stop=(dp == DCH // 2 - 1),
                                perf_mode=mybir.MatmulPerfMode.DoubleRow,
                            )
                        if (sc + (0 if off else 1)) % 2 == 0:
                            nc.vector.tensor_copy(out=dst[:, sc, off:off + w], in_=ps[:, :w])
                        else:
                            nc.scalar.copy(out=dst[:, sc, off:off + w], in_=ps[:, :w])
                v_sb[name] = dst

            # --- attention + epilogue, per q-block of 512 ---
            for qb in range(S // 512):
                qsl = slice(qb * 512, (qb + 1) * 512)
                # scores^T and exp: e^T[k, q] = exp(x_s[k,:] @ G[q,:] + ck)
                # with x_s^T itself as the stationary operand (no K tiles).
                e_sb = {}
                for si, name in enumerate(("a", "b")):
                    et = epool.tile([P, SCH, 512], FP8, tag=f"e{name.upper()}")
                    for kc in range(SCH):
                        ps = ps512.tile([P, 512], F32, tag="ps512")
                        for ep in range(DCH // 2):
                            esl = slice(2 * ep, 2 * ep + 2)
                            nc.tensor.matmul(
                                ps,
                                lhsT=x_sb[name][:, esl, kc * P:(kc + 1) * P],
                                rhs=gT_sb[:, esl, qsl],
                                start=(ep == 0),
                                stop=(ep == DCH // 2 - 1),
                                perf_mode=mybir.MatmulPerfMode.DoubleRow,
                            )
                        # exp(score/sqrt(D) + SCALE*b_q.W_k.x_s[k] - ESHIFT)
                        nc.scalar.activation(
                            out=et[:, kc, :], in_=ps, func=AF.Exp,
                            bias=ck_sb[:, si, kc:kc + 1], scale=SCALE,
                        )
                    e_sb[name] = et

                # stage 1: interp + layernorm -> z
                def stage1(qi):
                    qc = qb * 4 + qi
                    qs = slice(qi * P, (qi + 1) * P)

                    xc_t = opool.tile([P, DIM], BF16, tag="xc")
                    nc.gpsimd.dma_start(out=xc_t, in_=xcr[b, qc * P:(qc + 1) * P, :])

                    # interp psums, split 385/385 so every PE pass is longer
                    # than a LDWEIGHTS (135ns); h1 carries the ones columns
                    # -> row-sums at p1 col 383 (= v col 768)
                    pa = {}
                    for name in ("a", "b"):
                        p0 = ps512.tile([P, 512], F32, tag="ps512")
                        p1 = ps512.tile([P, 512], F32, tag="ps512")
                        for kp in range(SCH // 2):
                            ksl = slice(2 * kp, 2 * kp + 2)
                            nc.tensor.matmul(
                                p0[:, 0:385],
                                lhsT=e_sb[name][:, ksl, qs],
                                rhs=v_sb[name][:, ksl, 0:385],
                                start=(kp == 0),
                                stop=(kp == SCH // 2 - 1),
                                perf_mode=mybir.MatmulPerfMode.DoubleRow,
                            )
                        for kp in range(SCH // 2):
                            ksl = slice(2 * kp, 2 * kp + 2)
                            nc.tensor.matmul(
                                p1[:, 0:385],
                                lhsT=e_sb[name][:, ksl, qs],
                                rhs=v_sb[name][:, ksl, 385:VW],
                                start=(kp == 0),
                                stop=(kp == SCH // 2 - 1),
                                perf_mode=mybir.MatmulPerfMode.DoubleRow,
                            )
                        pa[name] = (p0, p1)

                    rcp = {}
                    for name in ("a", "b"):
                        r = spool.tile([P, 1], F32, tag=f"r{name}")
                        nc.vector.reciprocal(r, pa[name][1][:, 383:384])
                        rcp[name] = r

                    # t1 = psA*rA + xc ; t1 += psB*rB   (residual fused)
                    t1 = spool.tile([P, DIM], F32, tag="t1")
                    for (off, w, pi) in ((0, 385, 0), (385, 383, 1)):
                        nc.vector.scalar_tensor_tensor(
                            out=t1[:, off:off + w],
                            in0=pa["a"][pi][:, 0:w],
                            scalar=rcp["a"], in1=xc_t[:, off:off + w],
                            op0=ALU.mult, op1=ALU.add,
                        )
                        nc.vector.scalar_tensor_tensor(
                            out=t1[:, off:off + w],
                            in0=pa["b"][pi][:, 0:w],
                            scalar=rcp["b"], in1=t1[:, off:off + w],
                            op0=ALU.mult, op1=ALU.add,
                        )

                    # layernorm
                    stats = spool.tile([P, 3, 6], F32, tag="st")
                    for g in range(3):
                        nc.vector.bn_stats(
                            out=stats[:, g, :], in_=t1[:, g * 256:(g + 1) * 256]
                        )
                    mv = spool.tile([P, 2], F32, tag="mv")
                    nc.vector.bn_aggr(out=mv, in_=stats)
                    std = spool.tile([P, 1], F32, tag="std")
                    nc.scalar.activation(
                        out=std, in_=mv[:, 1:2], func=AF.Sqrt, bias=eps_sb
                    )
                    rstd = spool.tile([P, 1], F32, tag="rstd")
                    nc.vector.reciprocal(rstd, std)
                    # z = (t1 - mu) * rstd, computed on ACT as
                    # Identity(t1 * rstd + (-mu * rstd)); LayerNorm is now
                    # fully applied BEFORE fc, so the fc copyout is a plain
                    # +b_fc and no mean/std correction is needed after.
                    nmr = spool.tile([P, 1], F32, tag="nmr")
                    nc.vector.tensor_scalar(
                        nmr, mv[:, 0:1], -1.0, rstd, ALU.mult, ALU.mult
                    )
                    # z split DVE/ACT so the transposes (which wait on z)
                    # see ~half the latency
                    z = zpool.tile([P, DIM], BF16, tag="z")
                    nc.vector.tensor_scalar(
                        z[:, 0:256], t1[:, 0:256], rstd, nmr, ALU.mult, ALU.add
                    )
                    nc.scalar.activation(
                        out=z[:, 256:DIM], in_=t1[:, 256:DIM], func=AF.Identity,
                        bias=nmr, scale=rstd,
                    )
                    return z

                # cross-q-block pipeline: emit this block's interp/LN, and
                # between those groups drain the PREVIOUS block's stage-2
                # work (whose z is long since ready).
                for qi in range(4):
                    z = stage1(qi)
                    pending.append((b, qb * 4 + qi, z))
                    if len(pending) > 4:
                        stage2(*pending.pop(0))

        for args in pending:
            stage2(*args)

    nc.compile()
    return nc


_CACHED_NC = None
_LAST_IN_MAPS = None


def kernel(**inputs) -> np.ndarray:
    global _CACHED_NC, _LAST_IN_MAPS
    bf16 = ml_dtypes.bfloat16
    f32 = np.float32

    xA = np.asarray(inputs["x_A"], dtype=f32)
    xB = np.asarray(inputs["x_B"], dtype=f32)
    xC = np.asarray(inputs["x_C"], dtype=f32)

    fp8 = ml_dtypes.float8_e4m3
    xaT = np.ascontiguousarray(xA.transpose(0, 2, 1)).astype(fp8)
    xbT = np.ascontiguousarray(xB.transpose(0, 2, 1)).astype(fp8)
    xcT = np.ascontiguousarray(xC.transpose(0, 2, 1)).astype(fp8)
    xcr = (xC + 2.0 * np.asarray(inputs["b_v"], dtype=f32)).astype(bf16)

    W_q = np.asarray(inputs["W_q"], dtype=f32)
    W_k = np.asarray(inputs["W_k"], dtype=f32)
    b_q = np.asarray(inputs["b_q"], dtype=f32)
    # scores = Q K^T = x_C (W_q^T W_k) x_s^T + (b_q W_k) x_s^T
    wm = np.ascontiguousarray(W_q.T @ W_k).astype(fp8)
    vk = b_q @ W_k
    # per-key exp bias, laid out [b, p, src, kc] so the DMA is contiguous
    ck_full = np.stack(
        [SCALE * (xA @ vk) - ESHIFT, SCALE * (xB @ vk) - ESHIFT], axis=1
    )  # [B, 2, S]
    ck_arr = np.ascontiguousarray(
        ck_full.reshape(B, 2, S // P, P).transpose(0, 3, 1, 2)
    ).astype(f32)  # [B, P, 2, SCH]
    wvT = np.ascontiguousarray(np.asarray(inputs["W_v"], dtype=f32).T).astype(fp8)

    # fold LayerNorm's gamma/beta into the fc layer (exact):
    #   h = z*gamma + beta;  out = h @ W_fc.T + b_fc
    #     = z @ (W_fc * gamma).T + (b_fc + W_fc @ beta)
    gam = np.asarray(inputs["gamma"], dtype=f32)
    bet = np.asarray(inputs["beta"], dtype=f32)
    W_fc = np.asarray(inputs["W_fc"], dtype=f32)
    wfT = np.ascontiguousarray(W_fc.T * gam[:, None]).astype(bf16)
    bfc = (np.asarray(inputs["b_fc"], dtype=f32) + W_fc @ bet).astype(bf16)

    if _CACHED_NC is None:
        _CACHED_NC = build_bass()
    nc = _CACHED_NC

    in_maps = []
    for c in range(NCORES):
        sl = slice(c * BPC, (c + 1) * BPC)
        in_maps.append({
            "xaT": np.ascontiguousarray(xaT[sl]),
            "xbT": np.ascontiguousarray(xbT[sl]),
            "xcT": np.ascontiguousarray(xcT[sl]),
            "xcr": np.ascontiguousarray(xcr[sl]),
            "wm": wm, "wvT": wvT, "wfT": wfT,
            "ck": np.ascontiguousarray(ck_arr[sl]), "bfc": bfc,
        })

    _LAST_IN_MAPS = in_maps
    res = run_bass_kernel_spmd(nc, in_maps, core_ids=list(range(NCORES)))
    outs = [np.asarray(res.results[i]["out"], dtype=f32) for i in range(NCORES)]
    return np.concatenate(outs, axis=0)


if __name__ == "__main__":
    rng = np.random.default_rng(0)
    fake = {
        "x_A": rng.standard_normal((B, S, DIM), dtype=np.float32),
        "x_B": rng.standard_normal((B, S, DIM), dtype=np.float32),
        "x_C": rng.standard_normal((B, S, DIM), dtype=np.float32),
        "W_q": rng.standard_normal((DIM, DIM), dtype=np.float32) / 27.7,
        "b_q": rng.standard_normal(DIM).astype(np.float32) / 27.7,
        "W_k": rng.standard_normal((DIM, DIM), dtype=np.float32) / 27.7,
        "b_k": rng.standard_normal(DIM).astype(np.float32) / 27.7,
        "W_v": rng.standard_normal((DIM, DIM), dtype=np.float32) / 27.7,
        "b_v": rng.standard_normal(DIM).astype(np.float32) / 27.7,
        "gamma": np.ones(DIM, np.float32),
        "beta": np.zeros(DIM, np.float32),
        "W_fc": rng.standard_normal((DIM, DIM), dtype=np.float32) / 27.7,
        "b_fc": rng.standard_normal(DIM).astype(np.float32) / 27.7,
    }
    o = kernel(**fake)
    print(o.shape, o.dtype)

